# revision 1
# baseline (speedup 1.0000x reference)
"""Trainium2 distributed kernel for Swin-style attention with relative position bias.

Problem: nn_Attention_35450660061694
  B=32, N=576, DIM=768, H=12, D=64, TABLE=2209
  out = softmax(q@k^T * scale + bias_table[rel_index]) @ v @ w_out + b_out

Sharding: data-parallel over batch (4 batches/core on 8 cores).
The rel-position bias gather (the "sparse" part) is split across cores by
(j-tile, i) columns and AllGathered as exp(bias) in bf16.

Dataflow per core (all matmuls bf16, fp32 accumulation):
  - qkT projection: w_qkv-stationary => q^T,k^T land in [head-dim, seq] layout
    (what the attention matmuls need) without any transposes.
  - v projection: x-stationary => v lands in [seq, head-dim] layout.
  - dots^T[j,i] per (batch, head-pair): row-packed pair matmuls (K=64 each).
  - softmax: exp on ACT (scale folded in), multiply by gathered exp(bias) on
    DVE, denominators via tile_position-packed ones-matmuls on PE, reciprocal
    batched on DVE, broadcast of 1/s via K=2 matmul on PE.
  - attn@v: v-stationary col-packed pair matmuls -> out^T accumulates in PSUM.
  - projection: out^T tiles are directly the lhsT for out @ w_out.
"""

import math
import os
import sys

sys.path.insert(0, "/opt/trn_rl_repo")

import numpy as np

import concourse.bass as bass
import concourse.mybir as mybir
import concourse.tile as tile
from concourse import bacc
from concourse import library_config
from concourse.bass_utils import run_bass_kernel_spmd

# ---------------- problem constants ----------------
B, N, DIM = 32, 576, 768
HEADS, DHEAD = 12, 64
TABLE = 2209
SCALE = DHEAD ** -0.5

NCORES = 8
BPC = B // NCORES          # batches per core = 4
NT = 5                     # j-tiles (4 full 128 + 1 of 64)
JL = [128, 128, 128, 128, 64]   # j-tile lengths
TPAD = 2304                # padded table rows (18*128)
CT = NT * N                # 2880 gather "columns" (t,i)
CPC = CT // NCORES         # 360 columns per core
GCALLS = 6                 # indirect_copy calls per core
GINDS = 960                # indices per group per call (<=1024 ISA limit)
CPG = CPC // 8             # columns per 16-partition group = 45
VP = 2304                  # padded table rows
IH = 288                   # i-half width
HP = HEADS // 2            # 6 head pairs

F32 = mybir.dt.float32
BF16 = mybir.dt.bfloat16
I16 = mybir.dt.int16
U16 = mybir.dt.uint16

_CACHE = {}


def _build():
    nc = bacc.Bacc(
        "TRN2", target_bir_lowering=False, debug=False, num_devices=NCORES
    )

    # ---------------- I/O ----------------
    xT = nc.dram_tensor("xT", [DIM, BPC * N], F32, kind="ExternalInput")
    w_qkv = nc.dram_tensor("w_qkv", [DIM, 3 * DIM], F32, kind="ExternalInput")
    w_out = nc.dram_tensor("w_out", [DIM, DIM], F32, kind="ExternalInput")
    b_out = nc.dram_tensor("b_out", [1, DIM], F32, kind="ExternalInput")
    btab = nc.dram_tensor("btab", [TPAD, HEADS], F32, kind="ExternalInput")
    idx = nc.dram_tensor("idx", [128, GCALLS * GINDS // 16], U16, kind="ExternalInput")
    id32 = nc.dram_tensor("id32", [128, 128], F32, kind="ExternalInput")
    sel = nc.dram_tensor("sel", [2, 128], F32, kind="ExternalInput")
    onescol = nc.dram_tensor("onescol", [128, 1], F32, kind="ExternalInput")
    onesrow = nc.dram_tensor("onesrow", [1, 128], F32, kind="ExternalInput")
    out = nc.dram_tensor("out", [BPC, N, DIM], F32, kind="ExternalOutput")

    # internal DRAM
    piece = nc.dram_tensor("piece", [128, HEADS * CPC], BF16)
    ebt_ag = nc.dram_tensor("ebt_ag", [NCORES, 128, HEADS * CPC], BF16,
                            addr_space="Shared")
    kT_dram = nc.dram_tensor("kT_dram", [HP, 128, BPC * N], BF16)
    v_dram = nc.dram_tensor("v_dram", [BPC, NT, 128, HEADS * (DHEAD + 1)], BF16)

    NI = BPC * N  # 2304

    with tile.TileContext(nc, num_cores=NCORES) as tc:
        with (
            tc.tile_pool(name="persist", bufs=1) as pp,
            tc.tile_pool(name="workA", bufs=2) as wa,
            tc.tile_pool(name="psb", bufs=2, space="PSUM") as psb,
        ):
            # ---------- constants ----------
            sel_sb = pp.tile([2, 128], BF16, name="sel_sb", tag="sel")
            nc.gpsimd.dma_start(out=sel_sb[:], in_=sel[:])
            onec_sb = pp.tile([128, 1], BF16, name="onec_sb", tag="onec")
            nc.gpsimd.dma_start(out=onec_sb[:], in_=onescol[:])
            oner_sb = pp.tile([1, 128], BF16, name="oner_sb", tag="oner")
            nc.gpsimd.dma_start(out=oner_sb[:], in_=onesrow[:])
            bout_sb = pp.tile([1, DIM], BF16, name="bout_sb", tag="bout")
            nc.gpsimd.dma_start(out=bout_sb[:], in_=b_out[:])

            # one flat exp-bias tile: free = h*CT + (t*576 + i)
            ebp_all = pp.tile([128, HEADS * CT], BF16, name="ebp_all",
                              tag="ebp")
            qT_sb = []
            for hp in range(HP):
                q_ = pp.tile([128, NI], BF16, name=f"qT_{hp}", tag=f"qT_{hp}")
                qT_sb.append(q_)
            wv_sb = pp.tile([128, 6, DIM], BF16, name="wv_sb", tag="wv")
            for kt in range(6):
                nc.gpsimd.dma_start(
                    out=wv_sb[:, kt, :],
                    in_=w_qkv[kt * 128:(kt + 1) * 128, 2 * DIM:3 * DIM],
                )

            with (
                tc.tile_pool(name="xTpool", bufs=1) as xp,
            ):
                xT_sb = []
                for kt in range(6):
                    t_ = xp.tile([128, NI], BF16, name=f"xT_{kt}", tag=f"xT_{kt}")
                    nc.gpsimd.dma_start(
                        out=t_[:], in_=xT[kt * 128:(kt + 1) * 128, :]
                    )
                    xT_sb.append(t_)

                DBG_FAKE_EBP = os.environ.get("DBG_FAKE_EBP") == "1"
                if DBG_FAKE_EBP:
                    nc.vector.memset(ebp_all[:], 1.0)
                # ========== gather phase (scoped pool) ==========
                # exp-bias gather via indirect_copy: heads live on
                # partitions (table column h on partition 16g+h), each
                # 16-partition group gathers its share of (t,i) columns;
                # PE transposes flip [h, j] -> [j, h].
                with tc.tile_pool(name="gtemp", bufs=1) as gp:
                  if not DBG_FAKE_EBP:
                      idx_sb = gp.tile([128, GCALLS * GINDS // 16], U16,
                                       name="idx_sb", tag="idx")
                      nc.sync.dma_start(out=idx_sb[:], in_=idx[:])
                      id_sb = gp.tile([128, 128], BF16, name="id_sb", tag="id32")
                      nc.gpsimd.dma_start(out=id_sb[:], in_=id32[:])
                      # exp(bias_table) loaded as 18 x [128 rows, 12 heads]
                      bt2 = gp.tile([128, TPAD // 128, HEADS], F32,
                                    name="bt2", tag="bt2")
                      nc.sync.dma_start(
                          out=bt2[:],
                          in_=btab.ap().rearrange("(g p) h -> p g h", p=128),
                      )
                      eb2 = gp.tile([128, TPAD // 128, HEADS], BF16,
                                    name="eb2", tag="eb2")
                      nc.scalar.activation(
                          eb2[:], bt2[:], mybir.ActivationFunctionType.Exp
                      )
                      # on-chip transpose -> ett[h, v] = exp(btab)[v, h]
                      ett = gp.tile([16, VP], BF16, name="ett", tag="ett")
                      for g in range(TPAD // 128):
                          te_ps = psb.tile([16, 128], BF16, name="te_ps",
                                           tag="dots")
                          nc.tensor.transpose(
                              out=te_ps[:HEADS, :],
                              in_=eb2[:, g, :],
                              identity=id_sb[:, :],
                          )
                          nc.vector.tensor_copy(
                              out=ett[0:HEADS, g * 128:(g + 1) * 128],
                              in_=te_ps[:HEADS, :],
                          )
                      dtab = gp.tile([128, VP], BF16, name="dtab", tag="dtab")
                      for g in range(8):
                          nc.sync.dma_start(
                              out=dtab[16 * g:16 * (g + 1), :], in_=ett[:, :]
                          )
                      gout = gp.tile([128, 8 * CPG * 16], BF16, name="gout",
                                     tag="gout")
                      for cc in range(GCALLS):
                          nc.gpsimd.indirect_copy(
                              out=gout[:, cc * GINDS:(cc + 1) * GINDS],
                              data=dtab[:],
                              idxs=idx_sb[:, cc * (GINDS // 16):
                                          (cc + 1) * (GINDS // 16)],
                              i_know_ap_gather_is_preferred=True,
                          )
                  if not DBG_FAKE_EBP:
                      # transpose each [32 (2 groups), 128 j] window -> [j, h]
                      stg = gp.tile([128, CPC * 16], BF16, name="stg", tag="stg")
                      for w in range(CPG):
                          for k in range(4):
                              tr_ps = psb.tile([128, 32], BF16, name="tr_ps",
                                               tag="dots")
                              nc.tensor.transpose(
                                  out=tr_ps[:, :],
                                  in_=gout[32 * k:32 * (k + 1),
                                           w * 128:(w + 1) * 128],
                                  identity=id_sb[32 * k:32 * (k + 1),
                                                 32 * k:32 * (k + 1)],
                                  tile_position=(32 * k, 0),
                              )
                              ct0 = w * 8 + 2 * k
                              nc.vector.tensor_copy(
                                  out=stg[:].rearrange(
                                      "p (h c) -> p h c", h=16
                                  )[:, :, ct0:ct0 + 2],
                                  in_=tr_ps[:].rearrange(
                                      "p (c h) -> p h c", h=16
                                  ),
                              )
                      # piece[h, ct] <- stg (h-major; junk h>=12 at the end)
                      nc.sync.dma_start(
                          out=piece.ap(), in_=stg[:, 0:HEADS * CPC]
                      )
                      nc.gpsimd.collective_compute(
                          "AllGather",
                          mybir.AluOpType.bypass,
                          replica_groups=[list(range(NCORES))],
                          ins=[piece.ap().opt()],
                          outs=[ebt_ag.ap().opt()],
                      )
                      # assemble: flat contiguous slices per (h, rank)
                      for h in range(HEADS):
                          for r in range(NCORES):
                              nc.sync.dma_start(
                                  out=ebp_all[:, h * CT + r * CPC:
                                              h * CT + (r + 1) * CPC],
                                  in_=ebt_ag[r, :, h * CPC:(h + 1) * CPC],
                              )
                  if True:
                      # ---------- v projection (x-stationary), all batches ----
                      for b in range(BPC):
                          for t in range(NT):
                              jl = JL[t]
                              ps_v = psb.tile([128, DIM], F32, name="ps_v",
                                              tag="dots")
                              for kt in range(6):
                                  for s0 in (0, 512):
                                      sl = min(512, DIM - s0)
                                      nc.tensor.matmul(
                                          ps_v[:jl, s0:s0 + sl],
                                          xT_sb[kt][:, b * N + t * 128:
                                                    b * N + t * 128 + jl],
                                          wv_sb[:, kt, s0:s0 + sl],
                                          start=(kt == 0),
                                          stop=(kt == 5),
                                      )
                              vb = wa.tile([128, HEADS * (DHEAD + 1)], BF16,
                                           name="vb", tag="vbounce", bufs=2)
                              nc.vector.memset(vb[:jl, :], 1.0)
                              nc.scalar.copy(
                                  out=vb[:jl, :].rearrange(
                                      "p (h c) -> p h c", c=DHEAD + 1
                                  )[:, :, 0:DHEAD],
                                  in_=ps_v[:jl, :],
                              )
                              nc.sync.dma_start(out=v_dram[b, t, :jl, :],
                                                in_=vb[:jl, :])

                  # ========== qk^T projection (w-stationary) ==========
                  if True:
                      for hp in range(HP):
                          for part in range(2):      # 0 => q, 1 => k
                              wq_t = wa.tile([128, 6, 128], BF16, name="wq_t",
                                             tag="wqk", bufs=3)
                              nc.gpsimd.dma_start(
                                  out=wq_t[:],
                                  in_=w_qkv[:, part * DIM + hp * 128:
                                            part * DIM + hp * 128 + 128]
                                  .rearrange("(a p) c -> p a c", p=128),
                              )
                              for qu in range(4):
                                  ps_qk = psb.tile([128, NI // 4], F32,
                                                   name="ps_qk", tag="dots")
                                  for kt in range(6):
                                      for s0 in (0, 512):
                                          sl = min(512, NI // 4 - s0)
                                          nc.tensor.matmul(
                                              ps_qk[:, s0:s0 + sl],
                                              wq_t[:, kt, :],
                                              xT_sb[kt][:, qu * (NI // 4) + s0:
                                                        qu * (NI // 4) + s0 + sl],
                                              start=(kt == 0),
                                              stop=(kt == 5),
                                          )
                                  if part == 0:
                                      nc.scalar.copy(
                                          out=qT_sb[hp][:, qu * (NI // 4):
                                                        (qu + 1) * (NI // 4)],
                                          in_=ps_qk[:],
                                      )
                                  else:
                                      kb = wa.tile([128, NI // 4], BF16,
                                                   name="kb", tag="kbounce",
                                                   bufs=2)
                                      nc.scalar.copy(out=kb[:], in_=ps_qk[:])
                                      nc.sync.dma_start(
                                          out=kT_dram[hp, :, qu * (NI // 4):
                                                      (qu + 1) * (NI // 4)],
                                          in_=kb[:],
                                      )

            # ========== attention (workB reuses xT/gather space) ==========
            with (
                tc.tile_pool(name="workB", bufs=2) as wb,
            ):
              if os.environ.get("DBG_SKIP_ATTN") == "1":
                z = wb.tile([128, DIM], F32, name="z", tag="obounce")
                nc.vector.memset(z[:], 0.0)
                for b in range(BPC):
                    for t in range(NT):
                        nc.sync.dma_start(
                            out=out[b, t * 128:t * 128 + JL[t], :],
                            in_=z[:JL[t], :])
              else:
                LVL = int(os.environ.get("DBG_ATTN_LVL", "9"))
                wo_sb = wb.tile([128, 6, DIM], BF16, name="wo_sb", tag="wo",
                                bufs=1)
                for kt in range(6):
                    nc.gpsimd.dma_start(
                        out=wo_sb[:, kt, :],
                        in_=w_out[kt * 128:(kt + 1) * 128, :],
                    )
                for b in range(BPC):
                    v_t = wa.tile([128, NT, HEADS * (DHEAD + 1)], BF16,
                                  name="v_t", tag="v_t", bufs=1)
                    nc.sync.dma_start(
                        out=v_t[:],
                        in_=v_dram[b].rearrange("t p c -> p t c"),
                    )

                    # denominator rows collected h-major: head h lands on
                    # partition 32*(h//3), free slot (h%3)*N
                    s4 = wb.tile([128, 3 * N], F32, name="s4", tag="s4",
                                 bufs=2)
                    outTn = []

                    for hp in range(HP):
                        kT_t = wa.tile([128, N], BF16, name="kT_t",
                                       tag="kT_t", bufs=3)
                        nc.sync.dma_start(
                            out=kT_t[:], in_=kT_dram[hp, :, b * N:(b + 1) * N]
                        )
                        # one accumulator per head (row 64 = ones-column
                        # denominators); ih halves in separate PSUM banks
                        # (start=True clears the whole bank).
                        outT_h = [
                            psb.tile([128, 1024], F32, name=f"outT_h{_h}",
                                     tag=f"outT{_h}", bufs=1)
                            for _h in range(2)
                        ]
                        for t in range(NT):
                            jl = JL[t]
                            for ih in range(2):
                                # head pair row-packed; each head gets its
                                # own PSUM bank (h0 at 0, h1 at 512)
                                dots = psb.tile([128, 1024], F32,
                                                name="dots", tag="dots")
                                i0 = b * N + ih * IH
                                nc.tensor.matmul(
                                    dots[:jl, 0:IH],
                                    kT_t[0:64, t * 128:t * 128 + jl],
                                    qT_sb[hp][0:64, i0:i0 + IH],
                                    start=True, stop=True,
                                    tile_position=(0, 0),
                                )
                                nc.tensor.matmul(
                                    dots[:jl, 512:512 + IH],
                                    kT_t[64:128, t * 128:t * 128 + jl],
                                    qT_sb[hp][64:128, i0:i0 + IH],
                                    start=True, stop=True,
                                    tile_position=(64, 0),
                                )
                                attnm = wa.tile([128, 2 * IH], BF16,
                                                name="attnm", tag="attnm",
                                                bufs=6)
                                for hloc in range(2):
                                    nc.scalar.activation(
                                        attnm[:jl, hloc * IH:(hloc + 1) * IH],
                                        dots[:jl, hloc * 512:hloc * 512 + IH],
                                        mybir.ActivationFunctionType.Exp,
                                        scale=float(SCALE),
                                    )
                                for hloc in range(2):
                                    h = 2 * hp + hloc
                                    nc.vector.tensor_tensor(
                                        out=attnm[:jl, hloc * IH:
                                                  (hloc + 1) * IH],
                                        in0=attnm[:jl, hloc * IH:
                                                  (hloc + 1) * IH],
                                        in1=ebp_all[:jl, h * CT + t * N
                                                    + ih * IH:
                                                    h * CT + t * N
                                                    + (ih + 1) * IH],
                                        op=mybir.AluOpType.mult,
                                    )
                                # attn@v with ones-column appended to v:
                                # row 64 of each accumulator = denominators
                                o0 = ih * 512
                                for hloc in range(2 if LVL >= 2 else 0):
                                    h = 2 * hp + hloc
                                    nc.tensor.matmul(
                                        outT_h[hloc][0:65, o0:o0 + IH],
                                        v_t[:jl, t, h * 65:h * 65 + 65],
                                        attnm[:jl, hloc * IH:(hloc + 1) * IH],
                                        start=(t == 0), stop=(t == NT - 1),
                                    )
                        # evict denominators + unnormalized outT
                        DBGSKIP3 = LVL < 3
                        oT = wa.tile([128, N], BF16, name="oT", tag="outTun",
                                     bufs=10)
                        outTn.append(oT)
                        for parity in range(0 if DBGSKIP3 else 2):
                            h = 2 * hp + parity
                            for ih in range(2):
                                nc.vector.tensor_copy(
                                    out=s4[32 * (h // 3):32 * (h // 3) + 1,
                                           (h % 3) * N + ih * IH:
                                           (h % 3) * N + (ih + 1) * IH],
                                    in_=outT_h[parity][64:65,
                                                       ih * 512:ih * 512 + IH],
                                )
                                nc.scalar.copy(
                                    out=oT[parity * 64:parity * 64 + 64,
                                           ih * IH:(ih + 1) * IH],
                                    in_=outT_h[parity][0:64,
                                                       ih * 512:ih * 512 + IH],
                                )
                        if (os.environ.get("DBG_DUMP_S") == "1"
                                and b == 0 and hp == 0):
                            oTf = wb.tile([128, N], F32, name="oTf",
                                          tag="s4f", bufs=1)
                            for ihd in range(2):
                                nc.vector.tensor_copy(
                                    out=oTf[:, ihd * IH:(ihd + 1) * IH],
                                    in_=outT_h[0][:, ihd * 512:ihd * 512 + IH])
                            nc.sync.dma_start(out=out[3, 0:128, 0:N],
                                              in_=oTf[:, :])

                    if os.environ.get("DBG_DUMP_S") == "1" and b == 0:
                        s4f = wb.tile([128, 3 * N], F32, name="s4f", tag="s4f",
                                      bufs=1)
                        nc.vector.tensor_copy(out=s4f[:], in_=s4[:])
                        nc.sync.dma_start(out=out[0, 0:128, :],
                                          in_=s4f[:, 0:DIM])
                        nc.sync.dma_start(out=out[1, 0:128, :],
                                          in_=s4f[:, DIM:2 * DIM])
                        nc.sync.dma_start(out=out[2, 0:128, 0:1728 - 2 * DIM],
                                          in_=s4f[:, 2 * DIM:])
                    if LVL < 4:
                        continue
                    # batched reciprocal of the 12 denominator rows
                    s2 = wb.tile([96, HEADS * N // 96], F32, name="s2",
                                 tag="s2", bufs=2)
                    nc.sync.dma_start(out=s2[:], in_=s4[0:97:32, :])
                    r2 = wb.tile([96, HEADS * N // 96], F32, name="r2",
                                 tag="r2", bufs=2)
                    nc.vector.reciprocal(out=r2[:], in_=s2[:])
                    r2b = wb.tile([96, HEADS * N // 96], BF16, name="r2b",
                                  tag="r2b", bufs=2)
                    nc.vector.tensor_copy(out=r2b[:], in_=r2[:])
                    rb = wb.tile([2, HP * N], BF16, name="rb", tag="rb",
                                 bufs=1)
                    for hp in range(HP):
                        nc.sync.dma_start(
                            out=rb[:, hp * N:(hp + 1) * N],
                            in_=r2b[16 * hp:16 * hp + 16, :],
                        )

                    # normalize out^T by broadcast 1/s (K=2 matmul broadcast)
                    for hp in range(HP if LVL >= 5 else 0):
                        r_ps = psb.tile([128, N], F32, name="r_ps", tag="dots")
                        for s0 in (0, 512):
                            sl = min(512, N - s0)
                            nc.tensor.matmul(
                                r_ps[:, s0:s0 + sl],
                                sel_sb[:, :],
                                rb[:, hp * N + s0:hp * N + s0 + sl],
                                start=True, stop=True,
                            )
                        nc.vector.tensor_tensor(
                            out=outTn[hp][:],
                            in0=outTn[hp][:],
                            in1=r_ps[:],
                            op=mybir.AluOpType.mult,
                        )

                    # final projection: out[i, :] = b_out + outTn.T @ w_out
                    for t in range(NT if LVL >= 6 else 0):
                        jl = JL[t]
                        ps_o = psb.tile([128, DIM], F32, name="ps_o",
                                        tag="dots")
                        for s0 in (0, 512):
                            sl = min(512, DIM - s0)
                            nc.tensor.matmul(
                                ps_o[:jl, s0:s0 + sl],
                                oner_sb[:, 0:jl],
                                bout_sb[:, s0:s0 + sl],
                                start=True, stop=False,
                            )
                            for hp in range(HP):
                                nc.tensor.matmul(
                                    ps_o[:jl, s0:s0 + sl],
                                    outTn[hp][:, t * 128:t * 128 + jl],
                                    wo_sb[:, hp, s0:s0 + sl],
                                    start=False, stop=(hp == HP - 1),
                                )
                        ob = wb.tile([128, DIM], F32, name="ob", tag="obounce",
                                     bufs=2)
                        nc.scalar.copy(out=ob[:jl, :], in_=ps_o[:jl, :])
                        nc.sync.dma_start(
                            out=out[b, t * 128:t * 128 + jl, :], in_=ob[:jl, :]
                        )

    nc.compile()
    return nc


def _prep_inputs(x, w_qkv, w_out, b_out, bias_table, rel_index):
    x = np.asarray(x, np.float32)
    w_qkv = np.asarray(w_qkv, np.float32)
    w_out = np.asarray(w_out, np.float32)
    b_out = np.asarray(b_out, np.float32).reshape(1, DIM)
    bias_table = np.asarray(bias_table, np.float32)
    rel_index = np.asarray(rel_index)

    btab = np.zeros((TPAD, HEADS), np.float32)
    btab[:TABLE] = bias_table
    sel = np.zeros((2, 128), np.float32)
    sel[0, 0:64] = 1.0
    sel[1, 64:128] = 1.0
    onescol = np.ones((128, 1), np.float32)
    onesrow = np.ones((1, 128), np.float32)

    in_maps = []
    for c in range(NCORES):
        xT_c = np.ascontiguousarray(
            x[c * BPC:(c + 1) * BPC].transpose(2, 0, 1).reshape(DIM, BPC * N)
        )
        idx_c = np.zeros((128, GCALLS * GINDS // 16), np.uint16)
        for cc in range(GCALLS):
            for g in range(8):
                m = np.arange(GINDS)
                w = (cc * GINDS + m) // 128
                jl = (cc * GINDS + m) % 128
                ct = c * CPC + w * 8 + g
                t = ct // N
                i = ct % N
                j = t * 128 + jl
                vals = np.where(j < N, rel_index[i, np.minimum(j, N - 1)], 0)
                arr = vals.astype(np.uint16).reshape(GINDS // 16, 16).T
                idx_c[16 * g:16 * (g + 1),
                      cc * (GINDS // 16):(cc + 1) * (GINDS // 16)] = arr
        in_maps.append({
            "xT": xT_c,
            "w_qkv": w_qkv,
            "w_out": w_out,
            "b_out": b_out,
            "btab": btab,
            "idx": idx_c,
            "sel": sel,
            "id32": np.eye(128, dtype=np.float32),
            "onescol": onescol,
            "onesrow": onesrow,
        })
    return in_maps


def get_nc():
    if "nc" not in _CACHE:
        _CACHE["nc"] = _build()
    return _CACHE["nc"]


def run(inputs, trace=False, **kw):
    nc = get_nc()
    in_maps = _prep_inputs(**inputs)
    res = run_bass_kernel_spmd(
        nc, in_maps, core_ids=list(range(NCORES)), trace=trace, **kw
    )
    outs = np.concatenate([res.results[c]["out"] for c in range(NCORES)], axis=0)
    return outs, res


def kernel(**inputs):
    outs, _ = run(inputs, trace=False)
    return outs



# revision 2
# speedup vs baseline: 1.0989x; 1.0989x over previous
"""Trainium2 distributed kernel for Swin-style attention with relative position bias.

Problem: nn_Attention_35450660061694
  B=32, N=576, DIM=768, H=12, D=64, TABLE=2209
  out = softmax(q@k^T * scale + bias_table[rel_index]) @ v @ w_out + b_out

Sharding: data-parallel over batch (4 batches/core on 8 cores).

v2 vs baseline:
  - gather sharded by (slice, 36-col) blocks; 10 slice-aligned indirect_copy
    calls; 3 pipelined AllGathers so early attention tiles unblock sooner;
    8-DMA-per-chunk ebp assembly (vs 96 copies).
  - one fused exp per (b,hp,t): [jl, 4, 288] strided over both heads and both
    i-halves; one fused 4D DVE bias-multiply against resident h-major ebp.
  - PE transposes 4-packed per window (concurrent quadrants) into a bitcast
    PSUM slice; stg un-interleave copies alternate DVE/Act.
  - inputs pre-cast to bf16 on host; eviction copies spread across engines;
    PSUM: one rotating [128,2048] tile (4 banks) + outT 2x[128,1024].
  - softmax denominators ride the attn@v ones-column (row 64), staged via the
    baseline s4/s2/rb path, normalized with the K=2 select-matmul broadcast.
"""

import math
import os
import sys

sys.path.insert(0, "/opt/trn_rl_repo")

import numpy as np
import ml_dtypes

import concourse.bass as bass
import concourse.mybir as mybir
import concourse.tile as tile
from concourse import bacc
from concourse import library_config
from concourse.bass_utils import run_bass_kernel_spmd

# ---------------- problem constants ----------------
B, N, DIM = 32, 576, 768
HEADS, DHEAD = 12, 64
TABLE = 2209
SCALE = DHEAD ** -0.5

NCORES = 8
BPC = B // NCORES          # batches per core = 4
NT = 5                     # j-tiles (4 full 128 + 1 of 64)
JL = [128, 128, 128, 128, 64]
IH = 288                   # i-half width
HP = HEADS // 2            # 6 head pairs
NS = 2 * NT                # 10 (t, ih) slices
CSH = IH // NCORES         # 36 i-cols per rank per slice
TPAD = 2304                # padded table rows
VP = TPAD
GC = 10                    # indirect_copy calls (one (t,ih) slice each)
GINDS = 576                # indices per 16-partition group per call
LCT = 360                  # local (s,c) columns per core (10 slices x 36)
CHUNKS = ((0, 4), (4, 8), (8, 10))   # AllGather slice ranges
NI = BPC * N               # 2304
D1 = DHEAD + 1             # 65: v block width per head (ones col at 64)

F32 = mybir.dt.float32
BF16 = mybir.dt.bfloat16
U16 = mybir.dt.uint16

_CACHE = {}


def _build():
    nc = bacc.Bacc(
        "TRN2", target_bir_lowering=False, debug=False, num_devices=NCORES
    )

    # ---------------- I/O (x/weights pre-cast to bf16 on host) --------------
    xT = nc.dram_tensor("xT", [DIM, NI], BF16, kind="ExternalInput")
    w_qkv = nc.dram_tensor("w_qkv", [DIM, 3 * DIM], BF16, kind="ExternalInput")
    w_out = nc.dram_tensor("w_out", [DIM, DIM], BF16, kind="ExternalInput")
    b_out = nc.dram_tensor("b_out", [1, DIM], BF16, kind="ExternalInput")
    btab = nc.dram_tensor("btab", [TPAD, HEADS], F32, kind="ExternalInput")
    idx = nc.dram_tensor("idx", [128, GC * GINDS // 16], U16,
                         kind="ExternalInput")
    id32 = nc.dram_tensor("id32", [128, 128], BF16, kind="ExternalInput")
    sel2 = nc.dram_tensor("sel2", [2, 128], BF16, kind="ExternalInput")
    onesrow = nc.dram_tensor("onesrow", [1, 128], BF16, kind="ExternalInput")
    out = nc.dram_tensor("out", [BPC, N, DIM], F32, kind="ExternalOutput")

    # internal DRAM
    piece_d = [
        nc.dram_tensor(f"piece{k}", [128, (b - a) * HEADS * CSH], BF16)
        for k, (a, b) in enumerate(CHUNKS)
    ]
    ag_d = [
        nc.dram_tensor(f"ag{k}", [NCORES, 128, (b - a) * HEADS * CSH], BF16,
                       addr_space="Shared")
        for k, (a, b) in enumerate(CHUNKS)
    ]
    kT_dram = nc.dram_tensor("kT_dram", [HP, 128, NI], BF16)
    v_dram = nc.dram_tensor("v_dram", [BPC, NT, 128, HEADS * D1], BF16)

    with tile.TileContext(nc, num_cores=NCORES) as tc:
        with (
            tc.tile_pool(name="persist", bufs=1) as pp,
            tc.tile_pool(name="psA", bufs=1, space="PSUM") as psA,
            tc.tile_pool(name="psB", bufs=1, space="PSUM") as psB,
        ):
            # ---------- persistent constants ----------
            sel_sb = pp.tile([2, 128], BF16, name="sel_sb", tag="sel")
            nc.sync.dma_start(out=sel_sb[:], in_=sel2[:])
            oner_sb = pp.tile([1, 128], BF16, name="oner_sb", tag="oner")
            nc.sync.dma_start(out=oner_sb[:], in_=onesrow[:])
            bout_sb = pp.tile([1, DIM], BF16, name="bout_sb", tag="bout")
            nc.sync.dma_start(out=bout_sb[:], in_=b_out[:])

            # exp-bias in 3 chunk regions; region k: free = h*(ns*288) +
            # s_local*288 + c  (h-major within region)
            ebp_r = [
                pp.tile([128, HEADS * (b - a) * IH], BF16,
                        name=f"ebp{k}", tag=f"ebp{k}")
                for k, (a, b) in enumerate(CHUNKS)
            ]
            ebp_shc = [
                ebp_r[k][:, :].rearrange("p (h s c) -> p s h c",
                                         s=(b - a), c=IH)
                for k, (a, b) in enumerate(CHUNKS)
            ]
            qT_sb = [pp.tile([128, NI], BF16, name=f"qT_{hp}", tag=f"qT_{hp}")
                     for hp in range(HP)]

            with tc.tile_pool(name="xp", bufs=1) as xp:
                xT_sb = []
                for kt in range(6):
                    t_ = xp.tile([128, NI], BF16, name=f"xT_{kt}",
                                 tag=f"xT_{kt}")
                    nc.sync.dma_start(
                        out=t_[:], in_=xT[kt * 128:(kt + 1) * 128, :]
                    )
                    xT_sb.append(t_)

                # ========== gather pipeline (pool open until pieces done) ====
                with tc.tile_pool(name="gp", bufs=1) as gp:
                    idx_sb = gp.tile([128, GC * GINDS // 16], U16,
                                     name="idx_sb", tag="idx")
                    nc.sync.dma_start(out=idx_sb[:], in_=idx[:])
                    id_sb = gp.tile([128, 128], BF16, name="id_sb", tag="id")
                    nc.sync.dma_start(out=id_sb[:], in_=id32[:])
                    bt2 = gp.tile([128, TPAD // 128, HEADS], F32,
                                  name="bt2", tag="bt2")
                    nc.sync.dma_start(
                        out=bt2[:],
                        in_=btab.ap().rearrange("(g p) h -> p g h", p=128),
                    )
                    eb2 = gp.tile([128, TPAD // 128, HEADS], BF16,
                                  name="eb2", tag="eb2")
                    nc.scalar.activation(
                        eb2[:], bt2[:], mybir.ActivationFunctionType.Exp
                    )
                    # transpose exp-table -> ett[h, v]; replicate to dtab
                    ett = gp.tile([16, VP], BF16, name="ett", tag="ett")
                    for g in range(TPAD // 128):
                        tp_ = psA.tile([128, 2048], F32, name="te", tag="big")
                        te_ps = tp_[:, 0:64].bitcast(BF16)
                        nc.tensor.transpose(
                            out=te_ps[0:HEADS, :],
                            in_=eb2[:, g, :],
                            identity=id_sb[:, :],
                        )
                        nc.vector.tensor_copy(
                            out=ett[0:HEADS, g * 128:(g + 1) * 128],
                            in_=te_ps[0:HEADS, :],
                        )
                    dtab = gp.tile([128, VP], BF16, name="dtab", tag="dtab")
                    for g in range(8):
                        nc.sync.dma_start(
                            out=dtab[16 * g:16 * (g + 1), :], in_=ett[:, :]
                        )
                    gout = gp.tile([128, GC * GINDS], BF16, name="gout",
                                   tag="gout")
                    for cc in range(GC):
                        nc.gpsimd.indirect_copy(
                            out=gout[:, cc * GINDS:(cc + 1) * GINDS],
                            data=dtab[:],
                            idxs=idx_sb[:, cc * (GINDS // 16):
                                        (cc + 1) * (GINDS // 16)],
                            i_know_ap_gather_is_preferred=True,
                        )
                    # stg[jl, h(16), lct(360)]
                    stg = gp.tile([128, 16 * LCT], BF16, name="stg",
                                  tag="stg")
                    stg3 = stg[:, :].rearrange("p (h l) -> p h l", h=16)

                    def emit_transposes(w0, w1):
                        for w in range(w0, w1):
                            tb = psA.tile([128, 2048], F32, name="tr",
                                          tag="big")
                            tbf = tb[:, 0:64].bitcast(BF16)
                            for k in range(4):
                                nc.tensor.transpose(
                                    out=tbf[:, 32 * k:32 * (k + 1)],
                                    in_=gout[32 * k:32 * (k + 1),
                                             w * 128:(w + 1) * 128],
                                    identity=id_sb[32 * k:32 * (k + 1),
                                                   32 * k:32 * (k + 1)],
                                    tile_position=(32 * k, 0),
                                )
                            eng = (nc.vector.tensor_copy if w % 2 == 0
                                   else nc.scalar.copy)
                            eng(
                                out=stg3[:, :, w * 8:(w + 1) * 8].rearrange(
                                    "p h (k c) -> p h k c", c=2),
                                in_=tbf[:, :].rearrange(
                                    "p (k c h) -> p h k c", k=4, c=2),
                            )

                    def emit_chunk(k):
                        a, b = CHUNKS[k]
                        nc.sync.dma_start(
                            out=piece_d[k].ap(),
                            in_=stg3[:, 0:HEADS, a * CSH:b * CSH],
                        )
                        nc.gpsimd.collective_compute(
                            "AllGather",
                            mybir.AluOpType.bypass,
                            replica_groups=[list(range(NCORES))],
                            ins=[piece_d[k].ap().opt()],
                            outs=[ag_d[k].ap().opt()],
                        )
                        for r in range(NCORES):
                            nc.sync.dma_start(
                                out=ebp_r[k][:, :].rearrange(
                                    "p (x c) -> p x c", c=IH
                                )[:, :, r * CSH:(r + 1) * CSH],
                                in_=ag_d[k][r].rearrange(
                                    "p (x c) -> p x c", c=CSH),
                            )

                    # ========== v projection (scoped pool) ==========
                    with tc.tile_pool(name="wvp", bufs=1) as wvp:
                        wv_sb = wvp.tile([128, 6, DIM], BF16, name="wv_sb",
                                         tag="wv")
                        for kt in range(6):
                            nc.scalar.dma_start(
                                out=wv_sb[:, kt, :],
                                in_=w_qkv[kt * 128:(kt + 1) * 128,
                                          2 * DIM:3 * DIM],
                            )
                        for b in range(BPC):
                            for t in range(NT):
                                jl = JL[t]
                                ps_v = psA.tile([128, 2048], F32,
                                                name="ps_v", tag="big")
                                for kt in range(6):
                                    for s0 in (0, 512):
                                        sl_ = min(512, DIM - s0)
                                        nc.tensor.matmul(
                                            ps_v[:jl, s0:s0 + sl_],
                                            xT_sb[kt][:, b * N + t * 128:
                                                      b * N + t * 128 + jl],
                                            wv_sb[:, kt, s0:s0 + sl_],
                                            start=(kt == 0),
                                            stop=(kt == 5),
                                        )
                                vb = wvp.tile([128, HEADS * D1], BF16,
                                              name="vb", tag="vb", bufs=3)
                                nc.vector.memset(
                                    vb[:jl, :].rearrange(
                                        "p (h c) -> p h c", c=D1
                                    )[:, :, DHEAD:D1],
                                    1.0,
                                )
                                nc.scalar.copy(
                                    out=vb[:jl, :].rearrange(
                                        "p (h c) -> p h c", c=D1
                                    )[:, :, 0:DHEAD],
                                    in_=ps_v[:jl, 0:DIM],
                                )
                                nc.gpsimd.dma_start(
                                    out=v_dram[b, t, :jl, :],
                                    in_=vb[:jl, :])

                    # ========== qkT projection ==========
                    def qk_proj(hp):
                        for part in range(2):      # 0 => q, 1 => k
                            wq_t = xp.tile([128, 6, 128], BF16, name="wq_t",
                                           tag="wqk", bufs=2)
                            nc.scalar.dma_start(
                                out=wq_t[:],
                                in_=w_qkv[:, part * DIM + hp * 128:
                                          part * DIM + hp * 128 + 128]
                                .rearrange("(a p) c -> p a c", p=128),
                            )
                            for qp in range(2):    # qu pairs (0,1), (2,3)
                                ps_qk = psA.tile([128, 2048], F32,
                                                 name="ps_qk", tag="big")
                                for kt in range(6):
                                    for qi in range(2):
                                        qb = (2 * qp + qi) * N
                                        for s0 in (0, 512):
                                            sl_ = min(512, N - s0)
                                            nc.tensor.matmul(
                                                ps_qk[:, qi * 1024 + s0:
                                                      qi * 1024 + s0 + sl_],
                                                wq_t[:, kt, :],
                                                xT_sb[kt][:, qb + s0:
                                                          qb + s0 + sl_],
                                                start=(kt == 0),
                                                stop=(kt == 5),
                                            )
                                sre = ps_qk[:, :].rearrange(
                                    "p (q c) -> p q c", c=1024)[:, :, 0:N]
                                if part == 0:
                                    dsl = qT_sb[hp][:, 2 * qp * N:
                                                    (2 * qp + 2) * N]
                                    nc.vector.tensor_copy(
                                        out=dsl.rearrange(
                                            "p (q c) -> p q c", q=2),
                                        in_=sre)
                                else:
                                    kb = xp.tile([128, 2 * N], BF16,
                                                 name="kb", tag="kb",
                                                 bufs=2)
                                    nc.scalar.copy(
                                        out=kb[:, :].rearrange(
                                            "p (q c) -> p q c", q=2),
                                        in_=sre)
                                    nc.gpsimd.dma_start(
                                        out=kT_dram[hp, :, 2 * qp * N:
                                                    (2 * qp + 2) * N],
                                        in_=kb[:],
                                    )

                    for hp in range(HP):
                        qk_proj(hp)

                    # transposes + chunk pipeline (PE after qkT)
                    emit_transposes(0, 18)
                    emit_chunk(0)          # slices 0-3 (t0, t1)
                    emit_transposes(18, 36)
                    emit_chunk(1)          # slices 4-7 (t2, t3)
                    emit_transposes(36, 45)
                    emit_chunk(2)          # slices 8-9 (t4)

                # ========== attention ==========
                with tc.tile_pool(name="wk", bufs=1) as wk:
                    wo_sb = wk.tile([128, 6, DIM], BF16, name="wo_sb",
                                    tag="wo")
                    for kt in range(6):
                        nc.gpsimd.dma_start(
                            out=wo_sb[:, kt, :],
                            in_=w_out[kt * 128:(kt + 1) * 128, :],
                        )

                    def attention(b, hp, v_t, s4):
                        kT_t = wk.tile([128, N], BF16, name="kT_t",
                                       tag="kT_t", bufs=3)
                        nc.sync.dma_start(
                            out=kT_t[:],
                            in_=kT_dram[hp, :, b * N:(b + 1) * N])
                        oT = [
                            psB.tile([128, 1024], F32, name=f"oT{p}",
                                     tag=f"oT{p}")
                            for p in range(2)
                        ]
                        for t in range(NT):
                            jl = JL[t]
                            dots = psA.tile([128, 2048], F32, name="dots",
                                            tag="big")
                            for hloc in range(2):
                                for ih in range(2):
                                    nc.tensor.matmul(
                                        dots[:jl, (2 * ih + hloc) * 512:
                                             (2 * ih + hloc) * 512 + IH],
                                        kT_t[64 * hloc:64 * hloc + 64,
                                             t * 128:t * 128 + jl],
                                        qT_sb[hp][64 * hloc:64 * hloc + 64,
                                                  b * N + ih * IH:
                                                  b * N + (ih + 1) * IH],
                                        start=True, stop=True,
                                        tile_position=(64 * hloc, 0),
                                    )
                            attnm = wk.tile([128, 4 * IH], BF16,
                                            name="attnm", tag="attnm",
                                            bufs=3)
                            nc.scalar.activation(
                                attnm[:jl, :].rearrange(
                                    "p (x c) -> p x c", c=IH),
                                dots[:jl, :].rearrange(
                                    "p (x c) -> p x c", c=512)[:, :, 0:IH],
                                mybir.ActivationFunctionType.Exp,
                                scale=float(SCALE),
                            )
                            am4 = attnm[:jl, :].rearrange(
                                "p (i h c) -> p i h c", i=2, h=2)
                            nc.vector.tensor_tensor(
                                out=am4,
                                in0=am4,
                                in1=ebp_shc[0][
                                    :jl, 2 * t:2 * t + 2,
                                    2 * hp:2 * hp + 2, :],
                                op=mybir.AluOpType.mult,
                            )
                            for hloc in range(2):
                                for ih in range(2):
                                    nc.tensor.matmul(
                                        oT[hloc][0:D1,
                                                 ih * 512:ih * 512 + IH],
                                        v_t[:jl, t, (2 * hp + hloc) * D1:
                                            (2 * hp + hloc + 1) * D1],
                                        am4[:, ih, hloc, :],
                                        start=(t == 0), stop=(t == NT - 1),
                                    )
                        # evictions: out dims rows 0:64, denominator row 64
                        oT2 = wk.tile([128, N], BF16, name=f"oT2_{hp}",
                                      tag=f"oT2_{hp}", bufs=1)
                        nc.vector.tensor_copy(
                            out=oT2[0:64, :].rearrange(
                                "p (i c) -> p i c", c=IH),
                            in_=oT[0][0:64, :].rearrange(
                                "p (i c) -> p i c", c=512)[:, :, 0:IH],
                        )
                        nc.scalar.copy(
                            out=oT2[64:128, :].rearrange(
                                "p (i c) -> p i c", c=IH),
                            in_=oT[1][0:64, :].rearrange(
                                "p (i c) -> p i c", c=512)[:, :, 0:IH],
                        )
                        for hloc in range(2):
                            h = 2 * hp + hloc
                            dden = s4[32 * (h // 3):32 * (h // 3) + 1,
                                      (h % 3) * N:(h % 3) * N + N].rearrange(
                                "p (i c) -> p i c", c=IH)
                            sden = oT[hloc][64:65, :].rearrange(
                                "p (i c) -> p i c", c=512)[:, :, 0:IH]
                            if hloc == 0:
                                nc.scalar.copy(out=dden, in_=sden)
                            else:
                                nc.vector.tensor_copy(out=dden, in_=sden)
                        return oT2

                    def finish_batch(b, s4, oT2s):
                        s2 = wk.tile([96, HEADS * N // 96], F32, name="s2",
                                     tag="s2", bufs=1)
                        nc.sync.dma_start(out=s2[:], in_=s4[0:97:32, :])
                        r2 = wk.tile([96, HEADS * N // 96], F32, name="r2",
                                     tag="r2", bufs=1)
                        nc.vector.reciprocal(out=r2[:], in_=s2[:])
                        r2b = wk.tile([96, HEADS * N // 96], BF16,
                                      name="r2b", tag="r2b", bufs=1)
                        nc.vector.tensor_copy(out=r2b[:], in_=r2[:])
                        rb = wk.tile([2, HP * N], BF16, name="rb", tag="rb",
                                     bufs=1)
                        for hp in range(HP):
                            nc.sync.dma_start(
                                out=rb[:, hp * N:(hp + 1) * N],
                                in_=r2b[16 * hp:16 * hp + 16, :],
                            )
                        for hp in range(HP):
                            r_ps = psA.tile([128, 2048], F32, name="r_ps",
                                            tag="big")
                            for s0 in (0, 512):
                                sl_ = min(512, N - s0)
                                nc.tensor.matmul(
                                    r_ps[:, s0:s0 + sl_],
                                    sel_sb[:, :],
                                    rb[:, hp * N + s0:hp * N + s0 + sl_],
                                    start=True, stop=True,
                                )
                            nc.vector.tensor_tensor(
                                out=oT2s[hp][:],
                                in0=oT2s[hp][:],
                                in1=r_ps[:, 0:N],
                                op=mybir.AluOpType.mult,
                            )
                        for t in range(NT):
                            jl = JL[t]
                            ps_o = psA.tile([128, 2048], F32, name="ps_o",
                                            tag="big")
                            for s0 in (0, 512):
                                sl_ = min(512, DIM - s0)
                                nc.tensor.matmul(
                                    ps_o[:jl, s0:s0 + sl_],
                                    oner_sb[:, 0:jl],
                                    bout_sb[:, s0:s0 + sl_],
                                    start=True, stop=False,
                                )
                                for hp in range(HP):
                                    nc.tensor.matmul(
                                        ps_o[:jl, s0:s0 + sl_],
                                        oT2s[hp][:, t * 128:t * 128 + jl],
                                        wo_sb[:, hp, s0:s0 + sl_],
                                        start=False, stop=(hp == HP - 1),
                                    )
                            ob = wk.tile([128, DIM], F32, name="ob",
                                         tag="ob", bufs=1)
                            nc.scalar.copy(out=ob[:jl, :],
                                           in_=ps_o[:jl, 0:DIM])
                            nc.sync.dma_start(
                                out=out[b, t * 128:t * 128 + jl, :],
                                in_=ob[:jl, :],
                            )

                    for b in range(BPC):
                        v_t = wk.tile([128, NT, HEADS * D1], BF16,
                                      name=f"v_t{b}", tag="v_t", bufs=1)
                        nc.sync.dma_start(
                            out=v_t[:],
                            in_=v_dram[b].rearrange("t p c -> p t c"),
                        )
                        s4 = wk.tile([128, 3 * N], F32, name="s4",
                                     tag="s4", bufs=1)
                        oT2s = [attention(b, hp, v_t, s4)
                                for hp in range(HP)]
                        finish_batch(b, s4, oT2s)

    nc.compile()
    return nc


def _prep_inputs(x, w_qkv, w_out, b_out, bias_table, rel_index):
    x = np.asarray(x, np.float32)
    w_qkv = np.asarray(w_qkv, ml_dtypes.bfloat16)
    w_out = np.asarray(w_out, ml_dtypes.bfloat16)
    b_out = np.asarray(b_out, ml_dtypes.bfloat16).reshape(1, DIM)
    bias_table = np.asarray(bias_table, np.float32)
    rel_index = np.asarray(rel_index)

    btab = np.zeros((TPAD, HEADS), np.float32)
    btab[:TABLE] = bias_table
    sel2 = np.zeros((2, 128), ml_dtypes.bfloat16)
    sel2[0, 0:64] = 1.0
    sel2[1, 64:128] = 1.0
    onesrow = np.ones((1, 128), ml_dtypes.bfloat16)
    id32a = np.eye(128, dtype=ml_dtypes.bfloat16)

    # per (group g, item M): w = M//128; jl = M%128; lct = w*8+g;
    # s = lct//36; c = lct%36
    M = np.arange(GC * GINDS)
    w = M // 128
    jlv = M % 128

    in_maps = []
    for r in range(NCORES):
        xT_c = np.ascontiguousarray(
            x[r * BPC:(r + 1) * BPC].transpose(2, 0, 1).reshape(DIM, NI)
        ).astype(ml_dtypes.bfloat16)
        idx16 = np.zeros((128, GC * GINDS // 16), np.uint16)
        for g in range(8):
            lct = w * 8 + g
            s = lct // CSH
            c = lct % CSH
            i = (s % 2) * IH + CSH * r + c
            t = s // 2
            j = t * 128 + jlv
            vals = np.where(j < N, rel_index[i, np.minimum(j, N - 1)], 0)
            arr = vals.reshape(GC * GINDS // 16, 16).T.astype(np.uint16)
            idx16[16 * g:16 * (g + 1), :] = arr
        in_maps.append({
            "xT": xT_c,
            "w_qkv": w_qkv,
            "w_out": w_out,
            "b_out": b_out,
            "btab": btab,
            "idx": idx16,
            "id32": id32a,
            "sel2": sel2,
            "onesrow": onesrow,
        })
    return in_maps


def get_nc():
    if "nc" not in _CACHE:
        _CACHE["nc"] = _build()
    return _CACHE["nc"]


def run(inputs, trace=False, **kw):
    nc = get_nc()
    in_maps = _prep_inputs(**inputs)
    res = run_bass_kernel_spmd(
        nc, in_maps, core_ids=list(range(NCORES)), trace=trace, **kw
    )
    outs = np.concatenate([res.results[c]["out"] for c in range(NCORES)],
                          axis=0)
    return outs, res


def kernel(**inputs):
    outs, _ = run(inputs, trace=False)
    return outs


# revision 3
# speedup vs baseline: 1.1844x; 1.0778x over previous
"""Trainium2 distributed kernel for Swin-style attention with relative position bias.

Problem: nn_Attention_35450660061694
  B=32, N=576, DIM=768, H=12, D=64, TABLE=2209
  out = softmax(q@k^T * scale + bias_table[rel_index]) @ v @ w_out + b_out

Sharding: data-parallel over batch (4 batches/core on 8 cores).

v2 vs baseline:
  - gather sharded by (slice, 36-col) blocks; 10 slice-aligned indirect_copy
    calls; 3 pipelined AllGathers so early attention tiles unblock sooner;
    8-DMA-per-chunk ebp assembly (vs 96 copies).
  - one fused exp per (b,hp,t): [jl, 4, 288] strided over both heads and both
    i-halves; one fused 4D DVE bias-multiply against resident h-major ebp.
  - PE transposes 4-packed per window (concurrent quadrants) into a bitcast
    PSUM slice; stg un-interleave copies alternate DVE/Act.
  - inputs pre-cast to bf16 on host; eviction copies spread across engines;
    PSUM: one rotating [128,2048] tile (4 banks) + outT 2x[128,1024].
  - softmax denominators ride the attn@v ones-column (row 64), staged via the
    baseline s4/s2/rb path, normalized with the K=2 select-matmul broadcast.
"""

import math
import os
import sys

sys.path.insert(0, "/opt/trn_rl_repo")

import numpy as np
import ml_dtypes

import concourse.bass as bass
import concourse.mybir as mybir
import concourse.tile as tile
from concourse import bacc
from concourse import library_config
from concourse.bass_utils import run_bass_kernel_spmd

# ---------------- problem constants ----------------
B, N, DIM = 32, 576, 768
HEADS, DHEAD = 12, 64
TABLE = 2209
SCALE = DHEAD ** -0.5

NCORES = 8
BPC = B // NCORES          # batches per core = 4
NT = 5                     # j-tiles (4 full 128 + 1 of 64)
JL = [128, 128, 128, 128, 64]
IH = 288                   # i-half width
HP = HEADS // 2            # 6 head pairs
NS = 2 * NT                # 10 (t, ih) slices
CSH = IH // NCORES         # 36 i-cols per rank per slice
TPAD = 2304                # padded table rows
VP = TPAD
GC = 10                    # indirect_copy calls (one (t,ih) slice each)
GINDS = 576                # indices per 16-partition group per call
LCT = 360                  # local (s,c) columns per core (10 slices x 36)
CHUNKS = ((0, 4), (4, 8), (8, 10))   # AllGather slice ranges
NI = BPC * N               # 2304
D1 = DHEAD + 1             # 65: v block width per head (ones col at 64)

F32 = mybir.dt.float32
BF16 = mybir.dt.bfloat16
U16 = mybir.dt.uint16

_CACHE = {}


def _build():
    nc = bacc.Bacc(
        "TRN2", target_bir_lowering=False, debug=False, num_devices=NCORES
    )

    # ---------------- I/O (x/weights pre-cast to bf16 on host) --------------
    xT = nc.dram_tensor("xT", [DIM, NI], BF16, kind="ExternalInput")
    w_qkv = nc.dram_tensor("w_qkv", [DIM, 3 * DIM], BF16, kind="ExternalInput")
    w_out = nc.dram_tensor("w_out", [DIM, DIM], BF16, kind="ExternalInput")
    b_out = nc.dram_tensor("b_out", [1, DIM], BF16, kind="ExternalInput")
    btab = nc.dram_tensor("btab", [TPAD, HEADS], F32, kind="ExternalInput")
    idx = nc.dram_tensor("idx", [128, GC * GINDS // 16], U16,
                         kind="ExternalInput")
    id32 = nc.dram_tensor("id32", [128, 128], BF16, kind="ExternalInput")
    sel2 = nc.dram_tensor("sel2", [2, 128], BF16, kind="ExternalInput")
    onesrow = nc.dram_tensor("onesrow", [1, 128], BF16, kind="ExternalInput")
    out = nc.dram_tensor("out", [BPC, N, DIM], F32, kind="ExternalOutput")

    # internal DRAM
    piece_d = [
        nc.dram_tensor(f"piece{k}", [128, (b - a) * HEADS * CSH], BF16)
        for k, (a, b) in enumerate(CHUNKS)
    ]
    ag_d = [
        nc.dram_tensor(f"ag{k}", [NCORES, 128, (b - a) * HEADS * CSH], BF16,
                       addr_space="Shared")
        for k, (a, b) in enumerate(CHUNKS)
    ]
    kT_dram = nc.dram_tensor("kT_dram", [HP, 128, NI], BF16)
    v_dram = nc.dram_tensor("v_dram", [BPC, NT, 128, HEADS * D1], BF16)

    with tile.TileContext(nc, num_cores=NCORES) as tc:
        with (
            tc.tile_pool(name="persist", bufs=1) as pp,
            tc.tile_pool(name="psA", bufs=1, space="PSUM") as psA,
            tc.tile_pool(name="psB", bufs=1, space="PSUM") as psB,
        ):
            def pbig(i):
                return psA.tile([128, 1024], F32, name="bg",
                                tag=f"bg{i % 2}")

            # ---------- persistent constants ----------
            sel_sb = pp.tile([2, 128], BF16, name="sel_sb", tag="sel")
            nc.sync.dma_start(out=sel_sb[:], in_=sel2[:])
            oner_sb = pp.tile([1, 128], BF16, name="oner_sb", tag="oner")
            nc.sync.dma_start(out=oner_sb[:], in_=onesrow[:])
            bout_sb = pp.tile([1, DIM], BF16, name="bout_sb", tag="bout")
            nc.sync.dma_start(out=bout_sb[:], in_=b_out[:])

            # exp-bias in 3 chunk regions; region k: free = h*(ns*288) +
            # s_local*288 + c  (h-major within region)
            ebp_r = [
                pp.tile([128, HEADS * (b - a) * IH], BF16,
                        name=f"ebp{k}", tag=f"ebp{k}")
                for k, (a, b) in enumerate(CHUNKS)
            ]
            ebp_shc = [
                ebp_r[k][:, :].rearrange("p (h s c) -> p s h c",
                                         s=(b - a), c=IH)
                for k, (a, b) in enumerate(CHUNKS)
            ]
            qT_sb = [pp.tile([128, NI], BF16, name=f"qT_{hp}", tag=f"qT_{hp}")
                     for hp in range(HP)]

            with tc.tile_pool(name="xp", bufs=1) as xp:
                xT_sb = []
                for kt in range(6):
                    t_ = xp.tile([128, NI], BF16, name=f"xT_{kt}",
                                 tag=f"xT_{kt}")
                    nc.sync.dma_start(
                        out=t_[:], in_=xT[kt * 128:(kt + 1) * 128, :]
                    )
                    xT_sb.append(t_)

                # ========== gather pipeline (pool open until pieces done) ====
                with tc.tile_pool(name="gp", bufs=1) as gp:
                    idx_sb = gp.tile([128, GC * GINDS // 16], U16,
                                     name="idx_sb", tag="idx")
                    nc.sync.dma_start(out=idx_sb[:], in_=idx[:])
                    id_sb = gp.tile([128, 128], BF16, name="id_sb", tag="id")
                    nc.sync.dma_start(out=id_sb[:], in_=id32[:])
                    bt2 = gp.tile([128, TPAD // 128, HEADS], F32,
                                  name="bt2", tag="bt2")
                    nc.sync.dma_start(
                        out=bt2[:],
                        in_=btab.ap().rearrange("(g p) h -> p g h", p=128),
                    )
                    eb2 = gp.tile([128, TPAD // 128, HEADS], BF16,
                                  name="eb2", tag="eb2")
                    nc.scalar.activation(
                        eb2[:], bt2[:], mybir.ActivationFunctionType.Exp
                    )
                    # transpose exp-table -> ett[h, v]; replicate to dtab
                    ett = gp.tile([16, VP], BF16, name="ett", tag="ett")
                    for g in range(TPAD // 128):
                        tp_ = psA.tile([128, 2048], F32, name="te", tag="big")
                        te_ps = tp_[:, 0:64].bitcast(BF16)
                        nc.tensor.transpose(
                            out=te_ps[0:HEADS, :],
                            in_=eb2[:, g, :],
                            identity=id_sb[:, :],
                        )
                        nc.vector.tensor_copy(
                            out=ett[0:HEADS, g * 128:(g + 1) * 128],
                            in_=te_ps[0:HEADS, :],
                        )
                    dtab = gp.tile([128, VP], BF16, name="dtab", tag="dtab")
                    for g in range(8):
                        nc.sync.dma_start(
                            out=dtab[16 * g:16 * (g + 1), :], in_=ett[:, :]
                        )
                    gout = gp.tile([128, GC * GINDS], BF16, name="gout",
                                   tag="gout")
                    for cc in range(GC):
                        nc.gpsimd.indirect_copy(
                            out=gout[:, cc * GINDS:(cc + 1) * GINDS],
                            data=dtab[:],
                            idxs=idx_sb[:, cc * (GINDS // 16):
                                        (cc + 1) * (GINDS // 16)],
                            i_know_ap_gather_is_preferred=True,
                        )
                    # stg[jl, h(16), lct(360)]
                    stg = gp.tile([128, 16 * LCT], BF16, name="stg",
                                  tag="stg")
                    stg3 = stg[:, :].rearrange("p (h l) -> p h l", h=16)

                    def emit_transposes(w0, w1):
                        for w in range(w0, w1):
                            tb = psA.tile([128, 2048], F32, name="tr",
                                          tag="big")
                            tbf = tb[:, 0:64].bitcast(BF16)
                            for k in range(4):
                                nc.tensor.transpose(
                                    out=tbf[:, 32 * k:32 * (k + 1)],
                                    in_=gout[32 * k:32 * (k + 1),
                                             w * 128:(w + 1) * 128],
                                    identity=id_sb[32 * k:32 * (k + 1),
                                                   32 * k:32 * (k + 1)],
                                    tile_position=(32 * k, 0),
                                )
                            eng = (nc.vector.tensor_copy if w % 2 == 0
                                   else nc.scalar.copy)
                            eng(
                                out=stg3[:, :, w * 8:(w + 1) * 8].rearrange(
                                    "p h (k c) -> p h k c", c=2),
                                in_=tbf[:, :].rearrange(
                                    "p (k c h) -> p h k c", k=4, c=2),
                            )

                    def emit_chunk(k):
                        a, b = CHUNKS[k]
                        nc.sync.dma_start(
                            out=piece_d[k].ap(),
                            in_=stg3[:, 0:HEADS, a * CSH:b * CSH],
                        )
                        nc.gpsimd.collective_compute(
                            "AllGather",
                            mybir.AluOpType.bypass,
                            replica_groups=[list(range(NCORES))],
                            ins=[piece_d[k].ap().opt()],
                            outs=[ag_d[k].ap().opt()],
                        )
                        for r in range(NCORES):
                            nc.sync.dma_start(
                                out=ebp_r[k][:, :].rearrange(
                                    "p (x c) -> p x c", c=IH
                                )[:, :, r * CSH:(r + 1) * CSH],
                                in_=ag_d[k][r].rearrange(
                                    "p (x c) -> p x c", c=CSH),
                            )

                    # ========== v projection (scoped pool) ==========
                    with tc.tile_pool(name="wvp", bufs=1) as wvp:
                        wv_sb = wvp.tile([128, 6, DIM], BF16, name="wv_sb",
                                         tag="wv")
                        for kt in range(6):
                            nc.scalar.dma_start(
                                out=wv_sb[:, kt, :],
                                in_=w_qkv[kt * 128:(kt + 1) * 128,
                                          2 * DIM:3 * DIM],
                            )
                        for b in range(BPC):
                            for t in range(NT):
                                jl = JL[t]
                                ps_v = pbig(b * NT + t)
                                for kt in range(6):
                                    for s0 in (0, 512):
                                        sl_ = min(512, DIM - s0)
                                        nc.tensor.matmul(
                                            ps_v[:jl, s0:s0 + sl_],
                                            xT_sb[kt][:, b * N + t * 128:
                                                      b * N + t * 128 + jl],
                                            wv_sb[:, kt, s0:s0 + sl_],
                                            start=(kt == 0),
                                            stop=(kt == 5),
                                        )
                                vb = wvp.tile([128, HEADS * D1], BF16,
                                              name="vb", tag="vb", bufs=3)
                                nc.vector.memset(
                                    vb[:jl, :].rearrange(
                                        "p (h c) -> p h c", c=D1
                                    )[:, :, DHEAD:D1],
                                    1.0,
                                )
                                nc.scalar.copy(
                                    out=vb[:jl, :].rearrange(
                                        "p (h c) -> p h c", c=D1
                                    )[:, :, 0:DHEAD],
                                    in_=ps_v[:jl, 0:DIM],
                                )
                                nc.gpsimd.dma_start(
                                    out=v_dram[b, t, :jl, :],
                                    in_=vb[:jl, :])

                    # ========== qkT projection ==========
                    def qk_proj(hp):
                        for part in range(2):      # 0 => q, 1 => k
                            wq_t = xp.tile([128, 6, 128], BF16, name="wq_t",
                                           tag="wqk", bufs=2)
                            nc.scalar.dma_start(
                                out=wq_t[:],
                                in_=w_qkv[:, part * DIM + hp * 128:
                                          part * DIM + hp * 128 + 128]
                                .rearrange("(a p) c -> p a c", p=128),
                            )
                            for qp in range(2):    # qu pairs (0,1), (2,3)
                                tq = [pbig(0), pbig(1)]
                                for kt in range(6):
                                    for qi in range(2):
                                        qb = (2 * qp + qi) * N
                                        for s0 in (0, 512):
                                            sl_ = min(512, N - s0)
                                            nc.tensor.matmul(
                                                tq[qi][:, s0:s0 + sl_],
                                                wq_t[:, kt, :],
                                                xT_sb[kt][:, qb + s0:
                                                          qb + s0 + sl_],
                                                start=(kt == 0),
                                                stop=(kt == 5),
                                            )
                                for qi in range(2):
                                    qb = (2 * qp + qi) * N
                                    if part == 0:
                                        nc.vector.tensor_copy(
                                            out=qT_sb[hp][:, qb:qb + N],
                                            in_=tq[qi][:, 0:N])
                                    else:
                                        kb = xp.tile([128, N], BF16,
                                                     name="kb", tag="kb",
                                                     bufs=2)
                                        nc.scalar.copy(
                                            out=kb[:],
                                            in_=tq[qi][:, 0:N])
                                        nc.gpsimd.dma_start(
                                            out=kT_dram[hp, :, qb:qb + N],
                                            in_=kb[:],
                                        )

                    for hp in range(HP):
                        qk_proj(hp)

                    # transposes + chunk pipeline (PE after qkT)
                    emit_transposes(0, 18)
                    emit_chunk(0)          # slices 0-3 (t0, t1)
                    emit_transposes(18, 36)
                    emit_chunk(1)          # slices 4-7 (t2, t3)
                    emit_transposes(36, 45)
                    emit_chunk(2)          # slices 8-9 (t4)

                # ========== attention ==========
                with tc.tile_pool(name="wk", bufs=1) as wk:
                    wo_sb = wk.tile([128, 6, DIM], BF16, name="wo_sb",
                                    tag="wo")
                    for kt in range(6):
                        nc.gpsimd.dma_start(
                            out=wo_sb[:, kt, :],
                            in_=w_out[kt * 128:(kt + 1) * 128, :],
                        )

                    def attention(b, hp, v_t, s4):
                        kT_t = wk.tile([128, N], BF16, name="kT_t",
                                       tag="kT_t", bufs=3)
                        nc.sync.dma_start(
                            out=kT_t[:],
                            in_=kT_dram[hp, :, b * N:(b + 1) * N])
                        oT = [
                            psB.tile([128, 1024], F32, name=f"oT{p}",
                                     tag=f"oT{p}")
                            for p in range(2)
                        ]
                        for t in range(NT):
                            jl = JL[t]
                            dots = psA.tile([128, 2048], F32, name="dots",
                                            tag="big")
                            for hloc in range(2):
                                for ih in range(2):
                                    nc.tensor.matmul(
                                        dots[:jl, (2 * ih + hloc) * 512:
                                             (2 * ih + hloc) * 512 + IH],
                                        kT_t[64 * hloc:64 * hloc + 64,
                                             t * 128:t * 128 + jl],
                                        qT_sb[hp][64 * hloc:64 * hloc + 64,
                                                  b * N + ih * IH:
                                                  b * N + (ih + 1) * IH],
                                        start=True, stop=True,
                                        tile_position=(64 * hloc, 0),
                                    )
                            attnm = wk.tile([128, 4 * IH], BF16,
                                            name="attnm", tag="attnm",
                                            bufs=3)
                            nc.scalar.activation(
                                attnm[:jl, :].rearrange(
                                    "p (x c) -> p x c", c=IH),
                                dots[:jl, :].rearrange(
                                    "p (x c) -> p x c", c=512)[:, :, 0:IH],
                                mybir.ActivationFunctionType.Exp,
                                scale=float(SCALE),
                            )
                            am4 = attnm[:jl, :].rearrange(
                                "p (i h c) -> p i h c", i=2, h=2)
                            nc.vector.tensor_tensor(
                                out=am4,
                                in0=am4,
                                in1=ebp_shc[0][
                                    :jl, 2 * t:2 * t + 2,
                                    2 * hp:2 * hp + 2, :],
                                op=mybir.AluOpType.mult,
                            )
                            for hloc in range(2):
                                for ih in range(2):
                                    nc.tensor.matmul(
                                        oT[hloc][0:D1,
                                                 ih * 512:ih * 512 + IH],
                                        v_t[:jl, t, (2 * hp + hloc) * D1:
                                            (2 * hp + hloc + 1) * D1],
                                        am4[:, ih, hloc, :],
                                        start=(t == 0), stop=(t == NT - 1),
                                    )
                        # evictions: out dims rows 0:64, denominator row 64
                        oT2 = wk.tile([128, N], BF16, name=f"oT2_{hp}",
                                      tag=f"oT2_{hp}", bufs=1)
                        nc.vector.tensor_copy(
                            out=oT2[0:64, :].rearrange(
                                "p (i c) -> p i c", c=IH),
                            in_=oT[0][0:64, :].rearrange(
                                "p (i c) -> p i c", c=512)[:, :, 0:IH],
                        )
                        nc.scalar.copy(
                            out=oT2[64:128, :].rearrange(
                                "p (i c) -> p i c", c=IH),
                            in_=oT[1][0:64, :].rearrange(
                                "p (i c) -> p i c", c=512)[:, :, 0:IH],
                        )
                        for hloc in range(2):
                            h = 2 * hp + hloc
                            dden = s4[32 * (h // 3):32 * (h // 3) + 1,
                                      (h % 3) * N:(h % 3) * N + N].rearrange(
                                "p (i c) -> p i c", c=IH)
                            sden = oT[hloc][64:65, :].rearrange(
                                "p (i c) -> p i c", c=512)[:, :, 0:IH]
                            if hloc == 0:
                                nc.scalar.copy(out=dden, in_=sden)
                            else:
                                nc.vector.tensor_copy(out=dden, in_=sden)
                        return oT2

                    def finish_batch(b, s4, oT2s):
                        s2 = wk.tile([96, HEADS * N // 96], F32, name="s2",
                                     tag="s2", bufs=1)
                        nc.sync.dma_start(out=s2[:], in_=s4[0:97:32, :])
                        r2 = wk.tile([96, HEADS * N // 96], F32, name="r2",
                                     tag="r2", bufs=1)
                        nc.vector.reciprocal(out=r2[:], in_=s2[:])
                        r2b = wk.tile([96, HEADS * N // 96], BF16,
                                      name="r2b", tag="r2b", bufs=1)
                        nc.vector.tensor_copy(out=r2b[:], in_=r2[:])
                        rb = wk.tile([2, HP * N], BF16, name="rb", tag="rb",
                                     bufs=1)
                        for hp in range(HP):
                            nc.sync.dma_start(
                                out=rb[:, hp * N:(hp + 1) * N],
                                in_=r2b[16 * hp:16 * hp + 16, :],
                            )
                        for hp in range(HP):
                            r_ps = psA.tile([128, 2048], F32, name="r_ps",
                                            tag="big")
                            for s0 in (0, 512):
                                sl_ = min(512, N - s0)
                                nc.tensor.matmul(
                                    r_ps[:, s0:s0 + sl_],
                                    sel_sb[:, :],
                                    rb[:, hp * N + s0:hp * N + s0 + sl_],
                                    start=True, stop=True,
                                )
                            nc.vector.tensor_tensor(
                                out=oT2s[hp][:],
                                in0=oT2s[hp][:],
                                in1=r_ps[:, 0:N],
                                op=mybir.AluOpType.mult,
                            )
                        for t in range(NT):
                            jl = JL[t]
                            ps_o = psA.tile([128, 2048], F32, name="ps_o",
                                            tag="big")
                            for s0 in (0, 512):
                                sl_ = min(512, DIM - s0)
                                nc.tensor.matmul(
                                    ps_o[:jl, s0:s0 + sl_],
                                    oner_sb[:, 0:jl],
                                    bout_sb[:, s0:s0 + sl_],
                                    start=True, stop=False,
                                )
                                for hp in range(HP):
                                    nc.tensor.matmul(
                                        ps_o[:jl, s0:s0 + sl_],
                                        oT2s[hp][:, t * 128:t * 128 + jl],
                                        wo_sb[:, hp, s0:s0 + sl_],
                                        start=False, stop=(hp == HP - 1),
                                    )
                            ob = wk.tile([128, DIM], F32, name="ob",
                                         tag="ob", bufs=1)
                            nc.scalar.copy(out=ob[:jl, :],
                                           in_=ps_o[:jl, 0:DIM])
                            nc.sync.dma_start(
                                out=out[b, t * 128:t * 128 + jl, :],
                                in_=ob[:jl, :],
                            )

                    for b in range(BPC):
                        v_t = wk.tile([128, NT, HEADS * D1], BF16,
                                      name=f"v_t{b}", tag="v_t", bufs=1)
                        nc.sync.dma_start(
                            out=v_t[:],
                            in_=v_dram[b].rearrange("t p c -> p t c"),
                        )
                        s4 = wk.tile([128, 3 * N], F32, name="s4",
                                     tag="s4", bufs=1)
                        oT2s = [attention(b, hp, v_t, s4)
                                for hp in range(HP)]
                        finish_batch(b, s4, oT2s)

    nc.compile()
    return nc


def _prep_inputs(x, w_qkv, w_out, b_out, bias_table, rel_index):
    x = np.asarray(x, np.float32)
    w_qkv = np.asarray(w_qkv, ml_dtypes.bfloat16)
    w_out = np.asarray(w_out, ml_dtypes.bfloat16)
    b_out = np.asarray(b_out, ml_dtypes.bfloat16).reshape(1, DIM)
    bias_table = np.asarray(bias_table, np.float32)
    rel_index = np.asarray(rel_index)

    btab = np.zeros((TPAD, HEADS), np.float32)
    btab[:TABLE] = bias_table
    sel2 = np.zeros((2, 128), ml_dtypes.bfloat16)
    sel2[0, 0:64] = 1.0
    sel2[1, 64:128] = 1.0
    onesrow = np.ones((1, 128), ml_dtypes.bfloat16)
    id32a = np.eye(128, dtype=ml_dtypes.bfloat16)

    # per (group g, item M): w = M//128; jl = M%128; lct = w*8+g;
    # s = lct//36; c = lct%36
    M = np.arange(GC * GINDS)
    w = M // 128
    jlv = M % 128

    in_maps = []
    for r in range(NCORES):
        xT_c = np.ascontiguousarray(
            x[r * BPC:(r + 1) * BPC].transpose(2, 0, 1).reshape(DIM, NI)
        ).astype(ml_dtypes.bfloat16)
        idx16 = np.zeros((128, GC * GINDS // 16), np.uint16)
        for g in range(8):
            lct = w * 8 + g
            s = lct // CSH
            c = lct % CSH
            i = (s % 2) * IH + CSH * r + c
            t = s // 2
            j = t * 128 + jlv
            vals = np.where(j < N, rel_index[i, np.minimum(j, N - 1)], 0)
            arr = vals.reshape(GC * GINDS // 16, 16).T.astype(np.uint16)
            idx16[16 * g:16 * (g + 1), :] = arr
        in_maps.append({
            "xT": xT_c,
            "w_qkv": w_qkv,
            "w_out": w_out,
            "b_out": b_out,
            "btab": btab,
            "idx": idx16,
            "id32": id32a,
            "sel2": sel2,
            "onesrow": onesrow,
        })
    return in_maps


def get_nc():
    if "nc" not in _CACHE:
        _CACHE["nc"] = _build()
    return _CACHE["nc"]


def run(inputs, trace=False, **kw):
    nc = get_nc()
    in_maps = _prep_inputs(**inputs)
    res = run_bass_kernel_spmd(
        nc, in_maps, core_ids=list(range(NCORES)), trace=trace, **kw
    )
    outs = np.concatenate([res.results[c]["out"] for c in range(NCORES)],
                          axis=0)
    return outs, res


def kernel(**inputs):
    outs, _ = run(inputs, trace=False)
    return outs


# revision 4
# speedup vs baseline: 1.3460x; 1.1365x over previous
"""Trainium2 distributed kernel for Swin-style attention with relative position bias.

Problem: nn_Attention_35450660061694
  B=32, N=576, DIM=768, H=12, D=64, TABLE=2209
  out = softmax(q@k^T * scale + bias_table[rel_index]) @ v @ w_out + b_out

Sharding: data-parallel over batch (4 batches/core on 8 cores).

v2 vs baseline:
  - gather sharded by (slice, 36-col) blocks; 10 slice-aligned indirect_copy
    calls; 3 pipelined AllGathers so early attention tiles unblock sooner;
    8-DMA-per-chunk ebp assembly (vs 96 copies).
  - one fused exp per (b,hp,t): [jl, 4, 288] strided over both heads and both
    i-halves; one fused 4D DVE bias-multiply against resident h-major ebp.
  - PE transposes 4-packed per window (concurrent quadrants) into a bitcast
    PSUM slice; stg un-interleave copies alternate DVE/Act.
  - inputs pre-cast to bf16 on host; eviction copies spread across engines;
    PSUM: one rotating [128,2048] tile (4 banks) + outT 2x[128,1024].
  - softmax denominators ride the attn@v ones-column (row 64), staged via the
    baseline s4/s2/rb path, normalized with the K=2 select-matmul broadcast.
"""

import math
import os
import sys

sys.path.insert(0, "/opt/trn_rl_repo")

import numpy as np
import ml_dtypes

import concourse.bass as bass
import concourse.mybir as mybir
import concourse.tile as tile
from concourse import bacc
from concourse import library_config
from concourse.bass_utils import run_bass_kernel_spmd

# ---------------- problem constants ----------------
B, N, DIM = 32, 576, 768
HEADS, DHEAD = 12, 64
TABLE = 2209
SCALE = DHEAD ** -0.5

NCORES = 8
BPC = B // NCORES          # batches per core = 4
NT = 5                     # j-tiles (4 full 128 + 1 of 64)
JL = [128, 128, 128, 128, 64]
IH = 288                   # i-half width
HP = HEADS // 2            # 6 head pairs
NS = 2 * NT                # 10 (t, ih) slices
CSH = IH // NCORES         # 36 i-cols per rank per slice
TPAD = 2304                # padded table rows
VP = TPAD
GC = 10                    # indirect_copy calls (one (t,ih) slice each)
GINDS = 576                # indices per 16-partition group per call
LCT = 360                  # local (s,c) columns per core (10 slices x 36)
CHUNKS = ((0, 4), (4, 8), (8, 10))   # AllGather slice ranges
NI = BPC * N               # 2304
D1 = DHEAD + 1             # 65: v block width per head (ones col at 64)

F32 = mybir.dt.float32
BF16 = mybir.dt.bfloat16
U16 = mybir.dt.uint16

_CACHE = {}


def _build():
    nc = bacc.Bacc(
        "TRN2", target_bir_lowering=False, debug=False, num_devices=NCORES
    )

    # ---------------- I/O (x/weights pre-cast to bf16 on host) --------------
    xT = nc.dram_tensor("xT", [DIM, NI], BF16, kind="ExternalInput")
    w_qkv = nc.dram_tensor("w_qkv", [DIM, 3 * DIM], BF16, kind="ExternalInput")
    w_out = nc.dram_tensor("w_out", [DIM, DIM], BF16, kind="ExternalInput")
    b_out = nc.dram_tensor("b_out", [1, DIM], BF16, kind="ExternalInput")
    btab = nc.dram_tensor("btab", [TPAD, HEADS], F32, kind="ExternalInput")
    idx = nc.dram_tensor("idx", [128, GC * GINDS // 16], U16,
                         kind="ExternalInput")
    id32 = nc.dram_tensor("id32", [128, 128], BF16, kind="ExternalInput")
    sel2 = nc.dram_tensor("sel2", [2, 128], BF16, kind="ExternalInput")
    onesrow = nc.dram_tensor("onesrow", [1, 128], BF16, kind="ExternalInput")
    out = nc.dram_tensor("out", [BPC, N, DIM], F32, kind="ExternalOutput")

    # internal DRAM
    piece_d = [
        nc.dram_tensor(f"piece{k}", [128, (b - a) * HEADS * CSH], BF16)
        for k, (a, b) in enumerate(CHUNKS)
    ]
    ag_d = [
        nc.dram_tensor(f"ag{k}", [NCORES, 128, (b - a) * HEADS * CSH], BF16,
                       addr_space="Shared")
        for k, (a, b) in enumerate(CHUNKS)
    ]
    kT_dram = nc.dram_tensor("kT_dram", [HP, 128, NI], BF16)
    v_dram = nc.dram_tensor("v_dram", [BPC, NT, 128, HEADS * D1], BF16)

    with tile.TileContext(nc, num_cores=NCORES) as tc:
        with (
            tc.tile_pool(name="persist", bufs=1) as pp,
            tc.tile_pool(name="psA", bufs=1, space="PSUM") as psA,
            tc.tile_pool(name="psB", bufs=1, space="PSUM") as psB,
        ):
            def pbig(i):
                return psA.tile([128, 1024], F32, name="bg",
                                tag=f"bg{i % 2}")

            # ---------- persistent constants ----------
            sel_sb = pp.tile([2, 128], BF16, name="sel_sb", tag="sel")
            nc.sync.dma_start(out=sel_sb[:], in_=sel2[:])
            oner_sb = pp.tile([1, 128], BF16, name="oner_sb", tag="oner")
            nc.sync.dma_start(out=oner_sb[:], in_=onesrow[:])
            bout_sb = pp.tile([1, DIM], BF16, name="bout_sb", tag="bout")
            nc.sync.dma_start(out=bout_sb[:], in_=b_out[:])

            # exp-bias in 3 chunk regions; region k: free = h*(ns*288) +
            # s_local*288 + c  (h-major within region)
            ebp_r = [
                pp.tile([128, HEADS * (b - a) * IH], BF16,
                        name=f"ebp{k}", tag=f"ebp{k}")
                for k, (a, b) in enumerate(CHUNKS)
            ]
            ebp_shc = [
                ebp_r[k][:, :].rearrange("p (h s c) -> p s h c",
                                         s=(b - a), c=IH)
                for k, (a, b) in enumerate(CHUNKS)
            ]
            qT_sb = [pp.tile([128, NI], BF16, name=f"qT_{hp}", tag=f"qT_{hp}")
                     for hp in range(HP)]

            with tc.tile_pool(name="xp", bufs=1) as xp:
                xT_sb = []
                for kt in range(6):
                    t_ = xp.tile([128, NI], BF16, name=f"xT_{kt}",
                                 tag=f"xT_{kt}")
                    nc.sync.dma_start(
                        out=t_[:], in_=xT[kt * 128:(kt + 1) * 128, :]
                    )
                    xT_sb.append(t_)

                # ========== gather pipeline (pool open until pieces done) ====
                with tc.tile_pool(name="gp", bufs=1) as gp:
                    idx_sb = gp.tile([128, GC * GINDS // 16], U16,
                                     name="idx_sb", tag="idx")
                    nc.sync.dma_start(out=idx_sb[:], in_=idx[:])
                    id_sb = gp.tile([128, 128], BF16, name="id_sb", tag="id")
                    nc.sync.dma_start(out=id_sb[:], in_=id32[:])
                    bt2 = gp.tile([128, TPAD // 128, HEADS], F32,
                                  name="bt2", tag="bt2")
                    nc.sync.dma_start(
                        out=bt2[:],
                        in_=btab.ap().rearrange("(g p) h -> p g h", p=128),
                    )
                    eb2 = gp.tile([128, TPAD // 128, HEADS], BF16,
                                  name="eb2", tag="eb2")
                    nc.scalar.activation(
                        eb2[:], bt2[:], mybir.ActivationFunctionType.Exp
                    )
                    # transpose exp-table -> ett[h, v]; replicate to dtab
                    ett = gp.tile([16, VP], BF16, name="ett", tag="ett")
                    for g in range(TPAD // 128):
                        tp_ = psA.tile([128, 2048], F32, name="te", tag="big")
                        te_ps = tp_[:, 0:64].bitcast(BF16)
                        nc.tensor.transpose(
                            out=te_ps[0:HEADS, :],
                            in_=eb2[:, g, :],
                            identity=id_sb[:, :],
                        )
                        nc.vector.tensor_copy(
                            out=ett[0:HEADS, g * 128:(g + 1) * 128],
                            in_=te_ps[0:HEADS, :],
                        )
                    dtab = gp.tile([128, VP], BF16, name="dtab", tag="dtab")
                    for g in range(8):
                        nc.sync.dma_start(
                            out=dtab[16 * g:16 * (g + 1), :], in_=ett[:, :]
                        )
                    gout = gp.tile([128, GC * GINDS], BF16, name="gout",
                                   tag="gout")
                    for cc in range(GC):
                        nc.gpsimd.indirect_copy(
                            out=gout[:, cc * GINDS:(cc + 1) * GINDS],
                            data=dtab[:],
                            idxs=idx_sb[:, cc * (GINDS // 16):
                                        (cc + 1) * (GINDS // 16)],
                            i_know_ap_gather_is_preferred=True,
                        )
                    # stg[jl, h(16), lct(360)]
                    stg = gp.tile([128, 16 * LCT], BF16, name="stg",
                                  tag="stg")
                    stg3 = stg[:, :].rearrange("p (h l) -> p h l", h=16)

                    def emit_transposes(w0, w1):
                        for w in range(w0, w1):
                            tb = psA.tile([128, 2048], F32, name="tr",
                                          tag="big")
                            tbf = tb[:, 0:64].bitcast(BF16)
                            for k in range(4):
                                nc.tensor.transpose(
                                    out=tbf[:, 32 * k:32 * (k + 1)],
                                    in_=gout[32 * k:32 * (k + 1),
                                             w * 128:(w + 1) * 128],
                                    identity=id_sb[32 * k:32 * (k + 1),
                                                   32 * k:32 * (k + 1)],
                                    tile_position=(32 * k, 0),
                                )
                            eng = (nc.vector.tensor_copy if w % 2 == 0
                                   else nc.scalar.copy)
                            eng(
                                out=stg3[:, :, w * 8:(w + 1) * 8].rearrange(
                                    "p h (k c) -> p h k c", c=2),
                                in_=tbf[:, :].rearrange(
                                    "p (k c h) -> p h k c", k=4, c=2),
                            )

                    def emit_chunk(k):
                        a, b = CHUNKS[k]
                        nc.sync.dma_start(
                            out=piece_d[k].ap(),
                            in_=stg3[:, 0:HEADS, a * CSH:b * CSH],
                        )
                        nc.gpsimd.collective_compute(
                            "AllGather",
                            mybir.AluOpType.bypass,
                            replica_groups=[list(range(NCORES))],
                            ins=[piece_d[k].ap().opt()],
                            outs=[ag_d[k].ap().opt()],
                        )
                        for r in range(NCORES):
                            nc.sync.dma_start(
                                out=ebp_r[k][:, :].rearrange(
                                    "p (x c) -> p x c", c=IH
                                )[:, :, r * CSH:(r + 1) * CSH],
                                in_=ag_d[k][r].rearrange(
                                    "p (x c) -> p x c", c=CSH),
                            )

                    # ========== v projection (scoped pool) ==========
                    with tc.tile_pool(name="wvp", bufs=1) as wvp:
                        wv_sb = wvp.tile([128, 6, DIM], BF16, name="wv_sb",
                                         tag="wv")
                        for kt in range(6):
                            nc.scalar.dma_start(
                                out=wv_sb[:, kt, :],
                                in_=w_qkv[kt * 128:(kt + 1) * 128,
                                          2 * DIM:3 * DIM],
                            )
                        for b in range(BPC):
                            for t in range(NT):
                                jl = JL[t]
                                ps_v = pbig(b * NT + t)
                                for kt in range(6):
                                    for s0 in (0, 512):
                                        sl_ = min(512, DIM - s0)
                                        nc.tensor.matmul(
                                            ps_v[:jl, s0:s0 + sl_],
                                            xT_sb[kt][:, b * N + t * 128:
                                                      b * N + t * 128 + jl],
                                            wv_sb[:, kt, s0:s0 + sl_],
                                            start=(kt == 0),
                                            stop=(kt == 5),
                                        )
                                vb = wvp.tile([128, HEADS * D1], BF16,
                                              name="vb", tag="vb", bufs=3)
                                nc.vector.memset(
                                    vb[:jl, :].rearrange(
                                        "p (h c) -> p h c", c=D1
                                    )[:, :, DHEAD:D1],
                                    1.0,
                                )
                                nc.scalar.copy(
                                    out=vb[:jl, :].rearrange(
                                        "p (h c) -> p h c", c=D1
                                    )[:, :, 0:DHEAD],
                                    in_=ps_v[:jl, 0:DIM],
                                )
                                nc.scalar.dma_start(
                                    out=v_dram[b, t, :jl, :],
                                    in_=vb[:jl, :])

                    # ========== qkT projection ==========
                    def qk_proj(hp):
                        for part in range(2):      # 0 => q, 1 => k
                            wq_t = xp.tile([128, 6, 128], BF16, name="wq_t",
                                           tag="wqk", bufs=2)
                            nc.scalar.dma_start(
                                out=wq_t[:],
                                in_=w_qkv[:, part * DIM + hp * 128:
                                          part * DIM + hp * 128 + 128]
                                .rearrange("(a p) c -> p a c", p=128),
                            )
                            for qp in range(2):    # qu pairs (0,1), (2,3)
                                tq = [pbig(0), pbig(1)]
                                for kt in range(6):
                                    for qi in range(2):
                                        qb = (2 * qp + qi) * N
                                        for s0 in (0, 512):
                                            sl_ = min(512, N - s0)
                                            nc.tensor.matmul(
                                                tq[qi][:, s0:s0 + sl_],
                                                wq_t[:, kt, :],
                                                xT_sb[kt][:, qb + s0:
                                                          qb + s0 + sl_],
                                                start=(kt == 0),
                                                stop=(kt == 5),
                                            )
                                for qi in range(2):
                                    qb = (2 * qp + qi) * N
                                    if part == 0:
                                        nc.scalar.copy(
                                            out=qT_sb[hp][:, qb:qb + N],
                                            in_=tq[qi][:, 0:N])
                                    else:
                                        kb = xp.tile([128, N], BF16,
                                                     name="kb", tag="kb",
                                                     bufs=2)
                                        nc.scalar.copy(
                                            out=kb[:],
                                            in_=tq[qi][:, 0:N])
                                        nc.scalar.dma_start(
                                            out=kT_dram[hp, :, qb:qb + N],
                                            in_=kb[:],
                                        )

                    for hp in range(HP):
                        qk_proj(hp)

                    # transposes + chunk pipeline (PE after qkT)
                    emit_transposes(0, 18)
                    emit_chunk(0)          # slices 0-3 (t0, t1)
                    emit_transposes(18, 36)
                    emit_chunk(1)          # slices 4-7 (t2, t3)
                    emit_transposes(36, 45)
                    emit_chunk(2)          # slices 8-9 (t4)

                # ========== attention ==========
                with tc.tile_pool(name="wk", bufs=1) as wk:
                    wo_sb = wk.tile([128, 6, DIM], BF16, name="wo_sb",
                                    tag="wo")
                    for kt in range(6):
                        nc.gpsimd.dma_start(
                            out=wo_sb[:, kt, :],
                            in_=w_out[kt * 128:(kt + 1) * 128, :],
                        )

                    def attention(b, hp, v_t, s4):
                        kT_t = wk.tile([128, N], BF16, name="kT_t",
                                       tag="kT_t", bufs=3)
                        nc.scalar.dma_start(
                            out=kT_t[:],
                            in_=kT_dram[hp, :, b * N:(b + 1) * N])
                        oT = [
                            psB.tile([128, 1024], F32, name=f"oT{p}",
                                     tag=f"oT{p}")
                            for p in range(2)
                        ]
                        for t in range(NT):
                            jl = JL[t]
                            dots = psA.tile([128, 2048], F32, name="dots",
                                            tag="big")
                            for hloc in range(2):
                                for ih in range(2):
                                    nc.tensor.matmul(
                                        dots[:jl, (2 * ih + hloc) * 512:
                                             (2 * ih + hloc) * 512 + IH],
                                        kT_t[64 * hloc:64 * hloc + 64,
                                             t * 128:t * 128 + jl],
                                        qT_sb[hp][64 * hloc:64 * hloc + 64,
                                                  b * N + ih * IH:
                                                  b * N + (ih + 1) * IH],
                                        start=True, stop=True,
                                        tile_position=(64 * hloc, 0),
                                    )
                            attnm = wk.tile([128, 4 * IH], BF16,
                                            name="attnm", tag="attnm",
                                            bufs=3)
                            nc.scalar.activation(
                                attnm[:jl, :].rearrange(
                                    "p (x c) -> p x c", c=IH),
                                dots[:jl, :].rearrange(
                                    "p (x c) -> p x c", c=512)[:, :, 0:IH],
                                mybir.ActivationFunctionType.Exp,
                                scale=float(SCALE),
                            )
                            am4 = attnm[:jl, :].rearrange(
                                "p (i h c) -> p i h c", i=2, h=2)
                            nc.vector.tensor_tensor(
                                out=am4,
                                in0=am4,
                                in1=ebp_shc[0][
                                    :jl, 2 * t:2 * t + 2,
                                    2 * hp:2 * hp + 2, :],
                                op=mybir.AluOpType.mult,
                            )
                            for hloc in range(2):
                                for ih in range(2):
                                    nc.tensor.matmul(
                                        oT[hloc][0:D1,
                                                 ih * 512:ih * 512 + IH],
                                        v_t[:jl, t, (2 * hp + hloc) * D1:
                                            (2 * hp + hloc + 1) * D1],
                                        am4[:, ih, hloc, :],
                                        start=(t == 0), stop=(t == NT - 1),
                                    )
                        # evictions: out dims rows 0:64, denominator row 64
                        oT2 = wk.tile([128, N], BF16, name=f"oT2_{hp}",
                                      tag=f"oT2_{hp}", bufs=1)
                        nc.vector.tensor_copy(
                            out=oT2[0:64, :].rearrange(
                                "p (i c) -> p i c", c=IH),
                            in_=oT[0][0:64, :].rearrange(
                                "p (i c) -> p i c", c=512)[:, :, 0:IH],
                        )
                        nc.scalar.copy(
                            out=oT2[64:128, :].rearrange(
                                "p (i c) -> p i c", c=IH),
                            in_=oT[1][0:64, :].rearrange(
                                "p (i c) -> p i c", c=512)[:, :, 0:IH],
                        )
                        for hloc in range(2):
                            h = 2 * hp + hloc
                            dden = s4[32 * (h // 3):32 * (h // 3) + 1,
                                      (h % 3) * N:(h % 3) * N + N].rearrange(
                                "p (i c) -> p i c", c=IH)
                            sden = oT[hloc][64:65, :].rearrange(
                                "p (i c) -> p i c", c=512)[:, :, 0:IH]
                            if hloc == 0:
                                nc.scalar.copy(out=dden, in_=sden)
                            else:
                                nc.vector.tensor_copy(out=dden, in_=sden)
                        return oT2

                    def finish_batch(b, s4, oT2s):
                        s2 = wk.tile([96, HEADS * N // 96], F32, name="s2",
                                     tag="s2", bufs=1)
                        nc.sync.dma_start(out=s2[:], in_=s4[0:97:32, :])
                        r2 = wk.tile([96, HEADS * N // 96], F32, name="r2",
                                     tag="r2", bufs=1)
                        nc.vector.reciprocal(out=r2[:], in_=s2[:])
                        r2b = wk.tile([96, HEADS * N // 96], BF16,
                                      name="r2b", tag="r2b", bufs=1)
                        nc.vector.tensor_copy(out=r2b[:], in_=r2[:])
                        rb = wk.tile([2, HP * N], BF16, name="rb", tag="rb",
                                     bufs=1)
                        for hp in range(HP):
                            nc.sync.dma_start(
                                out=rb[:, hp * N:(hp + 1) * N],
                                in_=r2b[16 * hp:16 * hp + 16, :],
                            )
                        for hp in range(HP):
                            r_ps = psA.tile([128, 2048], F32, name="r_ps",
                                            tag="big")
                            for s0 in (0, 512):
                                sl_ = min(512, N - s0)
                                nc.tensor.matmul(
                                    r_ps[:, s0:s0 + sl_],
                                    sel_sb[:, :],
                                    rb[:, hp * N + s0:hp * N + s0 + sl_],
                                    start=True, stop=True,
                                )
                            nc.vector.tensor_tensor(
                                out=oT2s[hp][:],
                                in0=oT2s[hp][:],
                                in1=r_ps[:, 0:N],
                                op=mybir.AluOpType.mult,
                            )
                        for t in range(NT):
                            jl = JL[t]
                            ps_o = psA.tile([128, 2048], F32, name="ps_o",
                                            tag="big")
                            for s0 in (0, 512):
                                sl_ = min(512, DIM - s0)
                                nc.tensor.matmul(
                                    ps_o[:jl, s0:s0 + sl_],
                                    oner_sb[:, 0:jl],
                                    bout_sb[:, s0:s0 + sl_],
                                    start=True, stop=False,
                                )
                                for hp in range(HP):
                                    nc.tensor.matmul(
                                        ps_o[:jl, s0:s0 + sl_],
                                        oT2s[hp][:, t * 128:t * 128 + jl],
                                        wo_sb[:, hp, s0:s0 + sl_],
                                        start=False, stop=(hp == HP - 1),
                                    )
                            ob = wk.tile([128, DIM], F32, name="ob",
                                         tag="ob", bufs=1)
                            nc.scalar.copy(out=ob[:jl, :],
                                           in_=ps_o[:jl, 0:DIM])
                            nc.sync.dma_start(
                                out=out[b, t * 128:t * 128 + jl, :],
                                in_=ob[:jl, :],
                            )

                    for b in range(BPC):
                        v_t = wk.tile([128, NT, HEADS * D1], BF16,
                                      name=f"v_t{b}", tag="v_t", bufs=1)
                        nc.sync.dma_start(
                            out=v_t[:],
                            in_=v_dram[b].rearrange("t p c -> p t c"),
                        )
                        s4 = wk.tile([128, 3 * N], F32, name="s4",
                                     tag="s4", bufs=1)
                        oT2s = [attention(b, hp, v_t, s4)
                                for hp in range(HP)]
                        finish_batch(b, s4, oT2s)

    nc.compile()
    return nc


def _prep_inputs(x, w_qkv, w_out, b_out, bias_table, rel_index):
    x = np.asarray(x, np.float32)
    w_qkv = np.asarray(w_qkv, ml_dtypes.bfloat16)
    w_out = np.asarray(w_out, ml_dtypes.bfloat16)
    b_out = np.asarray(b_out, ml_dtypes.bfloat16).reshape(1, DIM)
    bias_table = np.asarray(bias_table, np.float32)
    rel_index = np.asarray(rel_index)

    btab = np.zeros((TPAD, HEADS), np.float32)
    btab[:TABLE] = bias_table
    sel2 = np.zeros((2, 128), ml_dtypes.bfloat16)
    sel2[0, 0:64] = 1.0
    sel2[1, 64:128] = 1.0
    onesrow = np.ones((1, 128), ml_dtypes.bfloat16)
    id32a = np.eye(128, dtype=ml_dtypes.bfloat16)

    # per (group g, item M): w = M//128; jl = M%128; lct = w*8+g;
    # s = lct//36; c = lct%36
    M = np.arange(GC * GINDS)
    w = M // 128
    jlv = M % 128

    in_maps = []
    for r in range(NCORES):
        xT_c = np.ascontiguousarray(
            x[r * BPC:(r + 1) * BPC].transpose(2, 0, 1).reshape(DIM, NI)
        ).astype(ml_dtypes.bfloat16)
        idx16 = np.zeros((128, GC * GINDS // 16), np.uint16)
        for g in range(8):
            lct = w * 8 + g
            s = lct // CSH
            c = lct % CSH
            i = (s % 2) * IH + CSH * r + c
            t = s // 2
            j = t * 128 + jlv
            vals = np.where(j < N, rel_index[i, np.minimum(j, N - 1)], 0)
            arr = vals.reshape(GC * GINDS // 16, 16).T.astype(np.uint16)
            idx16[16 * g:16 * (g + 1), :] = arr
        in_maps.append({
            "xT": xT_c,
            "w_qkv": w_qkv,
            "w_out": w_out,
            "b_out": b_out,
            "btab": btab,
            "idx": idx16,
            "id32": id32a,
            "sel2": sel2,
            "onesrow": onesrow,
        })
    return in_maps


def get_nc():
    if "nc" not in _CACHE:
        _CACHE["nc"] = _build()
    return _CACHE["nc"]


def run(inputs, trace=False, **kw):
    nc = get_nc()
    in_maps = _prep_inputs(**inputs)
    res = run_bass_kernel_spmd(
        nc, in_maps, core_ids=list(range(NCORES)), trace=trace, **kw
    )
    outs = np.concatenate([res.results[c]["out"] for c in range(NCORES)],
                          axis=0)
    return outs, res


def kernel(**inputs):
    outs, _ = run(inputs, trace=False)
    return outs


# revision 5
# speedup vs baseline: 1.3558x; 1.0073x over previous
"""Trainium2 distributed kernel for Swin-style attention with relative position bias.

Problem: nn_Attention_35450660061694
  B=32, N=576, DIM=768, H=12, D=64, TABLE=2209
  out = softmax(q@k^T * scale + bias_table[rel_index]) @ v @ w_out + b_out

Sharding: data-parallel over batch (4 batches/core on 8 cores).

v2 vs baseline:
  - gather sharded by (slice, 36-col) blocks; 10 slice-aligned indirect_copy
    calls; 3 pipelined AllGathers so early attention tiles unblock sooner;
    8-DMA-per-chunk ebp assembly (vs 96 copies).
  - one fused exp per (b,hp,t): [jl, 4, 288] strided over both heads and both
    i-halves; one fused 4D DVE bias-multiply against resident h-major ebp.
  - PE transposes 4-packed per window (concurrent quadrants) into a bitcast
    PSUM slice; stg un-interleave copies alternate DVE/Act.
  - inputs pre-cast to bf16 on host; eviction copies spread across engines;
    PSUM: one rotating [128,2048] tile (4 banks) + outT 2x[128,1024].
  - softmax denominators ride the attn@v ones-column (row 64), staged via the
    baseline s4/s2/rb path, normalized with the K=2 select-matmul broadcast.
"""

import math
import os
import sys

sys.path.insert(0, "/opt/trn_rl_repo")

import numpy as np
import ml_dtypes

import concourse.bass as bass
import concourse.mybir as mybir
import concourse.tile as tile
from concourse import bacc
from concourse import library_config
from concourse.bass_utils import run_bass_kernel_spmd

# ---------------- problem constants ----------------
B, N, DIM = 32, 576, 768
HEADS, DHEAD = 12, 64
TABLE = 2209
SCALE = DHEAD ** -0.5

NCORES = 8
BPC = B // NCORES          # batches per core = 4
NT = 5                     # j-tiles (4 full 128 + 1 of 64)
JL = [128, 128, 128, 128, 64]
IH = 288                   # i-half width
HP = HEADS // 2            # 6 head pairs
NS = 2 * NT                # 10 (t, ih) slices
CSH = IH // NCORES         # 36 i-cols per rank per slice
TPAD = 2304                # padded table rows
VP = TPAD
GC = 10                    # indirect_copy calls (one (t,ih) slice each)
GINDS = 576                # indices per 16-partition group per call
LCT = 360                  # local (s,c) columns per core (10 slices x 36)
CHUNKS = ((0, 4), (4, 8), (8, 10))   # AllGather slice ranges
NI = BPC * N               # 2304
D1 = DHEAD + 1             # 65: v block width per head (ones col at 64)

F32 = mybir.dt.float32
BF16 = mybir.dt.bfloat16
U16 = mybir.dt.uint16

_CACHE = {}


def _build():
    nc = bacc.Bacc(
        "TRN2", target_bir_lowering=False, debug=False, num_devices=NCORES
    )

    # ---------------- I/O (x/weights pre-cast to bf16 on host) --------------
    xT = nc.dram_tensor("xT", [DIM, NI], BF16, kind="ExternalInput")
    w_qkv = nc.dram_tensor("w_qkv", [DIM, 3 * DIM], BF16, kind="ExternalInput")
    w_out = nc.dram_tensor("w_out", [DIM, DIM], BF16, kind="ExternalInput")
    b_out = nc.dram_tensor("b_out", [1, DIM], BF16, kind="ExternalInput")
    btab = nc.dram_tensor("btab", [TPAD, HEADS], F32, kind="ExternalInput")
    idx = nc.dram_tensor("idx", [128, GC * GINDS // 16], U16,
                         kind="ExternalInput")
    id32 = nc.dram_tensor("id32", [128, 128], BF16, kind="ExternalInput")
    sel2 = nc.dram_tensor("sel2", [2, 128], BF16, kind="ExternalInput")
    onesrow = nc.dram_tensor("onesrow", [1, 128], BF16, kind="ExternalInput")
    out = nc.dram_tensor("out", [BPC, N, DIM], F32, kind="ExternalOutput")

    # internal DRAM
    piece_d = [
        nc.dram_tensor(f"piece{k}", [128, (b - a) * HEADS * CSH], BF16)
        for k, (a, b) in enumerate(CHUNKS)
    ]
    ag_d = [
        nc.dram_tensor(f"ag{k}", [NCORES, 128, (b - a) * HEADS * CSH], BF16,
                       addr_space="Shared")
        for k, (a, b) in enumerate(CHUNKS)
    ]
    kT_dram = nc.dram_tensor("kT_dram", [HP, 128, NI], BF16)
    v_dram = nc.dram_tensor("v_dram", [BPC, NT, 128, HEADS * D1], BF16)

    with tile.TileContext(nc, num_cores=NCORES) as tc:
        with (
            tc.tile_pool(name="persist", bufs=1) as pp,
            tc.tile_pool(name="psA", bufs=1, space="PSUM") as psA,
            tc.tile_pool(name="psB", bufs=1, space="PSUM") as psB,
        ):
            def pbig(i):
                return psA.tile([128, 1024], F32, name="bg",
                                tag=f"bg{i % 2}")

            # ---------- persistent constants ----------
            sel_sb = pp.tile([2, 128], BF16, name="sel_sb", tag="sel")
            nc.sync.dma_start(out=sel_sb[:], in_=sel2[:])
            oner_sb = pp.tile([1, 128], BF16, name="oner_sb", tag="oner")
            nc.sync.dma_start(out=oner_sb[:], in_=onesrow[:])
            bout_sb = pp.tile([1, DIM], BF16, name="bout_sb", tag="bout")
            nc.sync.dma_start(out=bout_sb[:], in_=b_out[:])

            # exp-bias in 3 chunk regions; region k: free = h*(ns*288) +
            # s_local*288 + c  (h-major within region)
            ebp_r = [
                pp.tile([128, HEADS * (b - a) * IH], BF16,
                        name=f"ebp{k}", tag=f"ebp{k}")
                for k, (a, b) in enumerate(CHUNKS)
            ]
            ebp_shc = [
                ebp_r[k][:, :].rearrange("p (h s c) -> p s h c",
                                         s=(b - a), c=IH)
                for k, (a, b) in enumerate(CHUNKS)
            ]
            qT_sb = [pp.tile([128, NI], BF16, name=f"qT_{hp}", tag=f"qT_{hp}")
                     for hp in range(HP)]

            with tc.tile_pool(name="xp", bufs=1) as xp:
                xT_sb = []
                for kt in range(6):
                    t_ = xp.tile([128, NI], BF16, name=f"xT_{kt}",
                                 tag=f"xT_{kt}")
                    nc.sync.dma_start(
                        out=t_[:], in_=xT[kt * 128:(kt + 1) * 128, :]
                    )
                    xT_sb.append(t_)

                # ========== gather pipeline (pool open until pieces done) ====
                with tc.tile_pool(name="gp", bufs=1) as gp:
                    idx_sb = gp.tile([128, GC * GINDS // 16], U16,
                                     name="idx_sb", tag="idx")
                    nc.sync.dma_start(out=idx_sb[:], in_=idx[:])
                    id_sb = gp.tile([128, 128], BF16, name="id_sb", tag="id")
                    nc.sync.dma_start(out=id_sb[:], in_=id32[:])
                    bt2 = gp.tile([128, TPAD // 128, HEADS], F32,
                                  name="bt2", tag="bt2")
                    nc.sync.dma_start(
                        out=bt2[:],
                        in_=btab.ap().rearrange("(g p) h -> p g h", p=128),
                    )
                    eb2 = gp.tile([128, TPAD // 128, HEADS], BF16,
                                  name="eb2", tag="eb2")
                    nc.scalar.activation(
                        eb2[:], bt2[:], mybir.ActivationFunctionType.Exp
                    )
                    # transpose exp-table -> ett[h, v]; replicate to dtab
                    ett = gp.tile([16, VP], BF16, name="ett", tag="ett")
                    for g in range(TPAD // 128):
                        tp_ = psA.tile([128, 2048], F32, name="te", tag="big")
                        te_ps = tp_[:, 0:64].bitcast(BF16)
                        nc.tensor.transpose(
                            out=te_ps[0:HEADS, :],
                            in_=eb2[:, g, :],
                            identity=id_sb[:, :],
                        )
                        nc.vector.tensor_copy(
                            out=ett[0:HEADS, g * 128:(g + 1) * 128],
                            in_=te_ps[0:HEADS, :],
                        )
                    dtab = gp.tile([128, VP], BF16, name="dtab", tag="dtab")
                    for g in range(8):
                        nc.sync.dma_start(
                            out=dtab[16 * g:16 * (g + 1), :], in_=ett[:, :]
                        )
                    gout = gp.tile([128, GC * GINDS], BF16, name="gout",
                                   tag="gout")
                    for cc in range(GC):
                        nc.gpsimd.indirect_copy(
                            out=gout[:, cc * GINDS:(cc + 1) * GINDS],
                            data=dtab[:],
                            idxs=idx_sb[:, cc * (GINDS // 16):
                                        (cc + 1) * (GINDS // 16)],
                            i_know_ap_gather_is_preferred=True,
                        )
                    # stg[jl, h(16), lct(360)]
                    stg = gp.tile([128, 16 * LCT], BF16, name="stg",
                                  tag="stg")
                    stg3 = stg[:, :].rearrange("p (h l) -> p h l", h=16)

                    def emit_transposes(w0, w1):
                        for w in range(w0, w1):
                            tb = psA.tile([128, 2048], F32, name="tr",
                                          tag="big")
                            tbf = tb[:, 0:64].bitcast(BF16)
                            for k in range(4):
                                nc.tensor.transpose(
                                    out=tbf[:, 32 * k:32 * (k + 1)],
                                    in_=gout[32 * k:32 * (k + 1),
                                             w * 128:(w + 1) * 128],
                                    identity=id_sb[32 * k:32 * (k + 1),
                                                   32 * k:32 * (k + 1)],
                                    tile_position=(32 * k, 0),
                                )
                            eng = (nc.vector.tensor_copy if w % 2 == 0
                                   else nc.scalar.copy)
                            eng(
                                out=stg3[:, :, w * 8:(w + 1) * 8].rearrange(
                                    "p h (k c) -> p h k c", c=2),
                                in_=tbf[:, :].rearrange(
                                    "p (k c h) -> p h k c", k=4, c=2),
                            )

                    def emit_chunk(k):
                        a, b = CHUNKS[k]
                        nc.sync.dma_start(
                            out=piece_d[k].ap(),
                            in_=stg3[:, 0:HEADS, a * CSH:b * CSH],
                        )
                        nc.gpsimd.collective_compute(
                            "AllGather",
                            mybir.AluOpType.bypass,
                            replica_groups=[list(range(NCORES))],
                            ins=[piece_d[k].ap().opt()],
                            outs=[ag_d[k].ap().opt()],
                        )
                        for r in range(NCORES):
                            nc.sync.dma_start(
                                out=ebp_r[k][:, :].rearrange(
                                    "p (x c) -> p x c", c=IH
                                )[:, :, r * CSH:(r + 1) * CSH],
                                in_=ag_d[k][r].rearrange(
                                    "p (x c) -> p x c", c=CSH),
                            )

                    # ========== v projection (scoped pool) ==========
                    with tc.tile_pool(name="wvp", bufs=1) as wvp:
                        wv_sb = wvp.tile([128, 6, DIM], BF16, name="wv_sb",
                                         tag="wv")
                        for kt in range(6):
                            nc.scalar.dma_start(
                                out=wv_sb[:, kt, :],
                                in_=w_qkv[kt * 128:(kt + 1) * 128,
                                          2 * DIM:3 * DIM],
                            )
                        for b in range(BPC):
                            for t in range(NT):
                                jl = JL[t]
                                ps_v = pbig(b * NT + t)
                                for kt in range(6):
                                    for s0 in (0, 512):
                                        sl_ = min(512, DIM - s0)
                                        nc.tensor.matmul(
                                            ps_v[:jl, s0:s0 + sl_],
                                            xT_sb[kt][:, b * N + t * 128:
                                                      b * N + t * 128 + jl],
                                            wv_sb[:, kt, s0:s0 + sl_],
                                            start=(kt == 0),
                                            stop=(kt == 5),
                                        )
                                vb = wvp.tile([128, HEADS * D1], BF16,
                                              name="vb", tag="vb", bufs=3)
                                nc.vector.memset(
                                    vb[:jl, :].rearrange(
                                        "p (h c) -> p h c", c=D1
                                    )[:, :, DHEAD:D1],
                                    1.0,
                                )
                                nc.scalar.copy(
                                    out=vb[:jl, :].rearrange(
                                        "p (h c) -> p h c", c=D1
                                    )[:, :, 0:DHEAD],
                                    in_=ps_v[:jl, 0:DIM],
                                )
                                nc.scalar.dma_start(
                                    out=v_dram[b, t, :jl, :],
                                    in_=vb[:jl, :])

                    # ========== qkT projection ==========
                    def qk_proj(hp):
                        for part in range(2):      # 0 => q, 1 => k
                            wq_t = xp.tile([128, 6, 128], BF16, name="wq_t",
                                           tag="wqk", bufs=2)
                            nc.scalar.dma_start(
                                out=wq_t[:],
                                in_=w_qkv[:, part * DIM + hp * 128:
                                          part * DIM + hp * 128 + 128]
                                .rearrange("(a p) c -> p a c", p=128),
                            )
                            for qp in range(2):    # qu pairs (0,1), (2,3)
                                tq = [pbig(0), pbig(1)]
                                for kt in range(6):
                                    for qi in range(2):
                                        qb = (2 * qp + qi) * N
                                        for s0 in (0, 512):
                                            sl_ = min(512, N - s0)
                                            nc.tensor.matmul(
                                                tq[qi][:, s0:s0 + sl_],
                                                wq_t[:, kt, :],
                                                xT_sb[kt][:, qb + s0:
                                                          qb + s0 + sl_],
                                                start=(kt == 0),
                                                stop=(kt == 5),
                                            )
                                for qi in range(2):
                                    qb = (2 * qp + qi) * N
                                    if part == 0:
                                        nc.scalar.copy(
                                            out=qT_sb[hp][:, qb:qb + N],
                                            in_=tq[qi][:, 0:N])
                                    else:
                                        kb = xp.tile([128, N], BF16,
                                                     name="kb", tag="kb",
                                                     bufs=2)
                                        nc.scalar.copy(
                                            out=kb[:],
                                            in_=tq[qi][:, 0:N])
                                        nc.scalar.dma_start(
                                            out=kT_dram[hp, :, qb:qb + N],
                                            in_=kb[:],
                                        )

                    for hp in range(HP):
                        qk_proj(hp)

                    # transposes + chunk pipeline (PE after qkT)
                    emit_transposes(0, 18)
                    emit_chunk(0)          # slices 0-3 (t0, t1)
                    emit_transposes(18, 36)
                    emit_chunk(1)          # slices 4-7 (t2, t3)
                    emit_transposes(36, 45)
                    emit_chunk(2)          # slices 8-9 (t4)

                # ========== attention ==========
                with tc.tile_pool(name="wk", bufs=1) as wk:
                    wo_sb = wk.tile([128, 6, DIM], BF16, name="wo_sb",
                                    tag="wo")
                    for kt in range(6):
                        nc.gpsimd.dma_start(
                            out=wo_sb[:, kt, :],
                            in_=w_out[kt * 128:(kt + 1) * 128, :],
                        )

                    def load_kt(b, hp):
                        kt = wk.tile([128, N], BF16, name="kT_t",
                                     tag="kT_t", bufs=3)
                        nc.scalar.dma_start(
                            out=kt[:],
                            in_=kT_dram[hp, :, b * N:(b + 1) * N])
                        return kt

                    def attention(b, hp, v_t, s4, kT_t):
                        oT = [
                            psB.tile([128, 1024], F32, name=f"oT{p}",
                                     tag=f"oT{p}")
                            for p in range(2)
                        ]
                        for t in range(NT):
                            jl = JL[t]
                            dots = psA.tile([128, 2048], F32, name="dots",
                                            tag="big")
                            for hloc in range(2):
                                for ih in range(2):
                                    nc.tensor.matmul(
                                        dots[:jl, (2 * ih + hloc) * 512:
                                             (2 * ih + hloc) * 512 + IH],
                                        kT_t[64 * hloc:64 * hloc + 64,
                                             t * 128:t * 128 + jl],
                                        qT_sb[hp][64 * hloc:64 * hloc + 64,
                                                  b * N + ih * IH:
                                                  b * N + (ih + 1) * IH],
                                        start=True, stop=True,
                                        tile_position=(64 * hloc, 0),
                                    )
                            attnm = wk.tile([128, 4 * IH], BF16,
                                            name="attnm", tag="attnm",
                                            bufs=3)
                            nc.scalar.activation(
                                attnm[:jl, :].rearrange(
                                    "p (x c) -> p x c", c=IH),
                                dots[:jl, :].rearrange(
                                    "p (x c) -> p x c", c=512)[:, :, 0:IH],
                                mybir.ActivationFunctionType.Exp,
                                scale=float(SCALE),
                            )
                            am4 = attnm[:jl, :].rearrange(
                                "p (i h c) -> p i h c", i=2, h=2)
                            nc.vector.tensor_tensor(
                                out=am4,
                                in0=am4,
                                in1=ebp_shc[0][
                                    :jl, 2 * t:2 * t + 2,
                                    2 * hp:2 * hp + 2, :],
                                op=mybir.AluOpType.mult,
                            )
                            for hloc in range(2):
                                for ih in range(2):
                                    nc.tensor.matmul(
                                        oT[hloc][0:D1,
                                                 ih * 512:ih * 512 + IH],
                                        v_t[:jl, t, (2 * hp + hloc) * D1:
                                            (2 * hp + hloc + 1) * D1],
                                        am4[:, ih, hloc, :],
                                        start=(t == 0), stop=(t == NT - 1),
                                    )
                        # evictions: out dims rows 0:64, denominator row 64
                        oT2 = wk.tile([128, N], BF16, name=f"oT2_{hp}",
                                      tag=f"oT2_{hp}", bufs=1)
                        nc.vector.tensor_copy(
                            out=oT2[0:64, :].rearrange(
                                "p (i c) -> p i c", c=IH),
                            in_=oT[0][0:64, :].rearrange(
                                "p (i c) -> p i c", c=512)[:, :, 0:IH],
                        )
                        nc.scalar.copy(
                            out=oT2[64:128, :].rearrange(
                                "p (i c) -> p i c", c=IH),
                            in_=oT[1][0:64, :].rearrange(
                                "p (i c) -> p i c", c=512)[:, :, 0:IH],
                        )
                        for hloc in range(2):
                            h = 2 * hp + hloc
                            dden = s4[32 * (h // 3):32 * (h // 3) + 1,
                                      (h % 3) * N:(h % 3) * N + N].rearrange(
                                "p (i c) -> p i c", c=IH)
                            sden = oT[hloc][64:65, :].rearrange(
                                "p (i c) -> p i c", c=512)[:, :, 0:IH]
                            if hloc == 0:
                                nc.scalar.copy(out=dden, in_=sden)
                            else:
                                nc.vector.tensor_copy(out=dden, in_=sden)
                        return oT2

                    def finish_batch(b, s4, oT2s):
                        s2 = wk.tile([96, HEADS * N // 96], F32, name="s2",
                                     tag="s2", bufs=1)
                        nc.sync.dma_start(out=s2[:], in_=s4[0:97:32, :])
                        r2 = wk.tile([96, HEADS * N // 96], F32, name="r2",
                                     tag="r2", bufs=1)
                        nc.vector.reciprocal(out=r2[:], in_=s2[:])
                        r2b = wk.tile([96, HEADS * N // 96], BF16,
                                      name="r2b", tag="r2b", bufs=1)
                        nc.vector.tensor_copy(out=r2b[:], in_=r2[:])
                        rb = wk.tile([2, HP * N], BF16, name="rb", tag="rb",
                                     bufs=1)
                        for hp in range(HP):
                            nc.sync.dma_start(
                                out=rb[:, hp * N:(hp + 1) * N],
                                in_=r2b[16 * hp:16 * hp + 16, :],
                            )
                        for hp in range(HP):
                            r_ps = psA.tile([128, 2048], F32, name="r_ps",
                                            tag="big")
                            for s0 in (0, 512):
                                sl_ = min(512, N - s0)
                                nc.tensor.matmul(
                                    r_ps[:, s0:s0 + sl_],
                                    sel_sb[:, :],
                                    rb[:, hp * N + s0:hp * N + s0 + sl_],
                                    start=True, stop=True,
                                )
                            nc.vector.tensor_tensor(
                                out=oT2s[hp][:],
                                in0=oT2s[hp][:],
                                in1=r_ps[:, 0:N],
                                op=mybir.AluOpType.mult,
                            )
                        for t in range(NT):
                            jl = JL[t]
                            ps_o = psA.tile([128, 2048], F32, name="ps_o",
                                            tag="big")
                            for s0 in (0, 512):
                                sl_ = min(512, DIM - s0)
                                nc.tensor.matmul(
                                    ps_o[:jl, s0:s0 + sl_],
                                    oner_sb[:, 0:jl],
                                    bout_sb[:, s0:s0 + sl_],
                                    start=True, stop=False,
                                )
                                for hp in range(HP):
                                    nc.tensor.matmul(
                                        ps_o[:jl, s0:s0 + sl_],
                                        oT2s[hp][:, t * 128:t * 128 + jl],
                                        wo_sb[:, hp, s0:s0 + sl_],
                                        start=False, stop=(hp == HP - 1),
                                    )
                            ob = wk.tile([128, DIM], F32, name="ob",
                                         tag="ob", bufs=1)
                            nc.scalar.copy(out=ob[:jl, :],
                                           in_=ps_o[:jl, 0:DIM])
                            nc.sync.dma_start(
                                out=out[b, t * 128:t * 128 + jl, :],
                                in_=ob[:jl, :],
                            )

                    for b in range(BPC):
                        v_t = wk.tile([128, NT, HEADS * D1], BF16,
                                      name=f"v_t{b}", tag="v_t", bufs=1)
                        nc.sync.dma_start(
                            out=v_t[:],
                            in_=v_dram[b].rearrange("t p c -> p t c"),
                        )
                        s4 = wk.tile([128, 3 * N], F32, name="s4",
                                     tag="s4", bufs=1)
                        kt_cur = load_kt(b, 0)
                        oT2s = []
                        for hp in range(HP):
                            kt_next = (load_kt(b, hp + 1)
                                       if hp + 1 < HP else None)
                            oT2s.append(attention(b, hp, v_t, s4, kt_cur))
                            kt_cur = kt_next
                        finish_batch(b, s4, oT2s)

    nc.compile()
    return nc


def _prep_inputs(x, w_qkv, w_out, b_out, bias_table, rel_index):
    x = np.asarray(x, np.float32)
    w_qkv = np.asarray(w_qkv, ml_dtypes.bfloat16)
    w_out = np.asarray(w_out, ml_dtypes.bfloat16)
    b_out = np.asarray(b_out, ml_dtypes.bfloat16).reshape(1, DIM)
    bias_table = np.asarray(bias_table, np.float32)
    rel_index = np.asarray(rel_index)

    btab = np.zeros((TPAD, HEADS), np.float32)
    btab[:TABLE] = bias_table
    sel2 = np.zeros((2, 128), ml_dtypes.bfloat16)
    sel2[0, 0:64] = 1.0
    sel2[1, 64:128] = 1.0
    onesrow = np.ones((1, 128), ml_dtypes.bfloat16)
    id32a = np.eye(128, dtype=ml_dtypes.bfloat16)

    # per (group g, item M): w = M//128; jl = M%128; lct = w*8+g;
    # s = lct//36; c = lct%36
    M = np.arange(GC * GINDS)
    w = M // 128
    jlv = M % 128

    in_maps = []
    for r in range(NCORES):
        xT_c = np.ascontiguousarray(
            x[r * BPC:(r + 1) * BPC].transpose(2, 0, 1).reshape(DIM, NI)
        ).astype(ml_dtypes.bfloat16)
        idx16 = np.zeros((128, GC * GINDS // 16), np.uint16)
        for g in range(8):
            lct = w * 8 + g
            s = lct // CSH
            c = lct % CSH
            i = (s % 2) * IH + CSH * r + c
            t = s // 2
            j = t * 128 + jlv
            vals = np.where(j < N, rel_index[i, np.minimum(j, N - 1)], 0)
            arr = vals.reshape(GC * GINDS // 16, 16).T.astype(np.uint16)
            idx16[16 * g:16 * (g + 1), :] = arr
        in_maps.append({
            "xT": xT_c,
            "w_qkv": w_qkv,
            "w_out": w_out,
            "b_out": b_out,
            "btab": btab,
            "idx": idx16,
            "id32": id32a,
            "sel2": sel2,
            "onesrow": onesrow,
        })
    return in_maps


def get_nc():
    if "nc" not in _CACHE:
        _CACHE["nc"] = _build()
    return _CACHE["nc"]


def run(inputs, trace=False, **kw):
    nc = get_nc()
    in_maps = _prep_inputs(**inputs)
    res = run_bass_kernel_spmd(
        nc, in_maps, core_ids=list(range(NCORES)), trace=trace, **kw
    )
    outs = np.concatenate([res.results[c]["out"] for c in range(NCORES)],
                          axis=0)
    return outs, res


def kernel(**inputs):
    outs, _ = run(inputs, trace=False)
    return outs
